# revision 22
# baseline (speedup 1.0000x reference)
"""Trainium2 Bass kernel for causal self-attention with RoPE (tensor-parallel over 8 cores).

Contract: kernel(**inputs) takes full unsharded inputs (x, W_attn, b_attn,
W_proj, b_proj), shards across 8 NeuronCores (2 heads each), runs one SPMD
Bass/Tile kernel, and host-reduces the partial c_proj outputs.

v2 design notes (vs baseline):
- RoPE entirely on DVE via partition-shifted scalar_tensor_tensor with a
  sign-folded sin table (no rotation matmuls, no scalar copies).
- Softmax denominator Z accumulated on DVE (f32) per key-block, finalized
  with a gpsimd partition_all_reduce (no [1,512] PE matmuls, no PSUM bank).
- Causal column restriction: diagonal key-blocks only compute score/exp/
  attV/Z columns >= c0; a single shared [128,128] triangle mask handles the
  block-diagonal boundary.
- Heads interleaved per q-tile with double-buffered y-PSUM; 1/Z fused into
  the y-PSUM evacuation (scalar_tensor_tensor).
- PSUM: qkv/v chains 2 banks, score pairs 2x[128,1024] 4 banks, y 2 banks.
"""

import os
import sys

import numpy as np

for _p in ("/opt/trn_rl_repo",):
    if os.path.isdir(_p) and _p not in sys.path:
        sys.path.insert(0, _p)

import ml_dtypes
from contextlib import ExitStack

import concourse.bass as bass
import concourse.tile as tile
from concourse import bacc, bass_isa, mybir
from concourse.bass_utils import run_bass_kernel_spmd

# ---- problem constants (hardcoded per contract) ----
B, T, C = 2, 2048, 2048
H, D = 16, 128
N_CORES = 8
HPC = H // N_CORES  # heads per core = 2
ROPE_BASE = 10000.0
SCALE = float(1.0 / np.sqrt(D))
TQ = 512            # query tile (free dim of scores matmul)
NTQ = T // TQ       # 4
TK = 128            # key tile (partition dim of scoresT)
NTK = T // TK       # 16
NCT = C // 128      # 16 contraction tiles for projections
BT = B * T
HD = D // 2         # rope half

F32 = mybir.dt.float32
BF16 = mybir.dt.bfloat16

ADD = mybir.AluOpType.add
MULT = mybir.AluOpType.mult
EXP = mybir.ActivationFunctionType.Exp

PAIR_LOOKAHEAD = 2  # score-pairs ahead of attV in the attention pipeline


def _build_program(with_bias_qk: bool, with_bias_v: bool):
    nc = bacc.Bacc(
        "TRN2", target_bir_lowering=False, debug=False, num_devices=N_CORES
    )

    xT = nc.dram_tensor("xT", [C, BT], BF16, kind="ExternalInput").ap()
    wqk = nc.dram_tensor("wqk", [128, NCT, 4 * D], BF16, kind="ExternalInput").ap()
    wv = nc.dram_tensor("wv", [128, NCT, HPC * D], BF16, kind="ExternalInput").ap()
    wpr = nc.dram_tensor("wpr", [128, HPC, C], BF16, kind="ExternalInput").ap()
    bqk = nc.dram_tensor("bqk", [128, 4], F32, kind="ExternalInput").ap()
    bqkr = nc.dram_tensor("bqkr", [128, 4], F32, kind="ExternalInput").ap()
    bv = nc.dram_tensor("bv", [HPC * D], F32, kind="ExternalInput").ap()
    cosT = nc.dram_tensor("cosT", [D, T], F32, kind="ExternalInput").ap()
    sinNT = nc.dram_tensor("sinNT", [D, T], F32, kind="ExternalInput").ap()
    tri = nc.dram_tensor("tri", [128, 128], BF16, kind="ExternalInput").ap()
    out = nc.dram_tensor("out", [BT, C], BF16, kind="ExternalOutput").ap()

    HT = T // 2  # half-batch token span (xT streamed in halves)

    with tile.TileContext(nc) as tc, ExitStack() as ctx:
        consts = ctx.enter_context(tc.tile_pool(name="consts", bufs=1))
        xt_pool = ctx.enter_context(tc.tile_pool(name="xt", bufs=2))
        qk_pool = ctx.enter_context(tc.tile_pool(name="qk", bufs=1))
        v_pool = ctx.enter_context(tc.tile_pool(name="v", bufs=1))
        e_pool = ctx.enter_context(tc.tile_pool(name="e", bufs=6))
        r_pool = ctx.enter_context(tc.tile_pool(name="rp", bufs=2))
        z_pool = ctx.enter_context(tc.tile_pool(name="zs", bufs=3))
        yn_pool = ctx.enter_context(tc.tile_pool(name="yn", bufs=1))
        ob_pool = ctx.enter_context(tc.tile_pool(name="ob", bufs=3))
        # PSUM: ps_s 2x[128,1024] (scores pairs / qkv chains / cproj) = 4
        # banks; ps_y 2x[128,512] attention-y only = 2 banks; ps_mm
        # 2x[128,256] V chains = 1-2 banks.
        ps_s = ctx.enter_context(tc.tile_pool(name="ps_s", bufs=2, space="PSUM"))
        ps_y = ctx.enter_context(tc.tile_pool(name="ps_y", bufs=2, space="PSUM"))
        ps_mm = ctx.enter_context(tc.tile_pool(name="ps_mm", bufs=2, space="PSUM"))

        qs = [nc.sync, nc.gpsimd, nc.scalar]
        wqk_sb = consts.tile([128, NCT, 4 * D], BF16)
        for i, sl in enumerate((slice(0, 5), slice(5, 10), slice(10, 16))):
            qs[i].dma_start(wqk_sb[:, sl, :], wqk[:, sl, :])

        def load_xt_half(b, h):
            xt_sb = xt_pool.tile([128, NCT, HT], BF16, tag="xt", name="xt_sb")
            for ct in range(NCT):
                qs[ct % 3].dma_start(
                    xt_sb[:, ct, :],
                    xT[ct * 128 : (ct + 1) * 128,
                       b * T + h * HT : b * T + (h + 1) * HT],
                )
            return xt_sb

        xt_cur = [load_xt_half(0, 0), load_xt_half(0, 1)]

        cos_sb = consts.tile([128, T], F32)
        nc.sync.dma_start(cos_sb[:], cosT[:])
        sin_sb = consts.tile([128, T], F32)
        nc.gpsimd.dma_start(sin_sb[:], sinNT[:])
        tri_sb = consts.tile([128, 128], BF16)
        nc.scalar.dma_start(tri_sb[:], tri[:])
        ones_sb = consts.tile([128, 1], BF16)
        nc.vector.memset(ones_sb[:], 1.0)
        wv_sb = consts.tile([128, NCT, HPC * D], BF16)
        nc.scalar.dma_start(wv_sb[:], wv[:])
        wpr_sb = consts.tile([128, HPC, C], BF16)
        nc.sync.dma_start(wpr_sb[:], wpr[:])
        if with_bias_qk:
            bqk_sb = consts.tile([128, 4], F32)
            nc.gpsimd.dma_start(bqk_sb[:], bqk[:])
            bqkr_sb = consts.tile([128, 4], F32)
            nc.gpsimd.dma_start(bqkr_sb[:], bqkr[:])
        if with_bias_v:
            bv_sb = consts.tile([128, HPC * D], F32)
            nc.gpsimd.dma_start(bv_sb[:], bv.to_broadcast((128, HPC * D)))

        def emit_rope(f, t, w, ps, qk_tiles):
            """Matmul-free rope over w cols starting at q-tile t:
            qk[f][:, tsl] = (q+b)*cos + rot_half(q+b)*sinN.
            Three DVE passes read the chain psum (shifted-base psum reads
            are legal); the final add runs on gpsimd."""
            tsl = slice(t * TQ, t * TQ + w)
            b_all = bqk_sb[:, f : f + 1] if with_bias_qk else 0.0
            b_lo = bqk_sb[0:HD, f : f + 1] if with_bias_qk else 0.0
            b_hi = bqk_sb[HD:D, f : f + 1] if with_bias_qk else 0.0
            t1 = r_pool.tile([128, 2 * TQ], BF16, tag="r1")
            nc.vector.scalar_tensor_tensor(
                t1[:, 0:w], ps[:, 0:w], b_all, cos_sb[:, tsl], op0=ADD, op1=MULT
            )
            t2 = r_pool.tile([128, 2 * TQ], BF16, tag="r2")
            nc.vector.scalar_tensor_tensor(
                t2[0:HD, 0:w], ps[HD:D, 0:w], b_hi, sin_sb[0:HD, tsl],
                op0=ADD, op1=MULT,
            )
            nc.vector.scalar_tensor_tensor(
                t2[HD:D, 0:w], ps[0:HD, 0:w], b_lo, sin_sb[HD:D, tsl],
                op0=ADD, op1=MULT,
            )
            nc.gpsimd.tensor_add(qk_tiles[f][:, tsl], t1[:, 0:w], t2[:, 0:w])

        def qkv_phase(b, xt_halves):
            """QKV projections + RoPE for batch b. Returns (qk_tiles, v_sb)."""
            qk_tiles = [
                qk_pool.tile([128, T], BF16, tag=f"qk{f}", name=f"qkt{f}")
                for f in range(4)
            ]
            if b == 0:
                # cold start: t=0 for all four f-tiles ct-major so the PE
                # consumes xT strips as the initial DMAs land.
                cold_a = ps_s.tile([128, 2 * TQ], F32, tag="s", name="cold_a")
                cold_b = ps_s.tile([128, 2 * TQ], F32, tag="s", name="cold_b")
                t0_ps = [
                    cold_a[:, 0:TQ], cold_a[:, TQ : 2 * TQ],
                    cold_b[:, 0:TQ], cold_b[:, TQ : 2 * TQ],
                ]
                for ct in range(NCT):
                    for f in range(4):
                        nc.tensor.matmul(
                            t0_ps[f],
                            wqk_sb[:, ct, f * 128 : (f + 1) * 128],
                            xt_halves[0][:, ct, 0:TQ],
                            start=(ct == 0),
                            stop=(ct == NCT - 1),
                        )
                for f in range(4):
                    emit_rope(f, 0, TQ, t0_ps[f], qk_tiles)
                spans = [(1, 1, 0), (2, 2, 1)]
            else:
                spans = [(0, 2, 0), (2, 2, 1)]
            # remaining q/k chains: wide psums (up to 32 matmuls each,
            # consecutive same-stationary) for ample rotation slack.
            for f in range(4):
                for t0, nt, h in spans:
                    w = nt * TQ
                    xt_sb = xt_halves[h]
                    ps = ps_s.tile([128, 2 * TQ], F32, tag="s", name="qkps")
                    for ct in range(NCT):
                        for u in range(nt):
                            lc = (t0 + u) * TQ - h * HT
                            nc.tensor.matmul(
                                ps[:, u * TQ : (u + 1) * TQ],
                                wqk_sb[:, ct, f * 128 : (f + 1) * 128],
                                xt_sb[:, ct, lc : lc + TQ],
                                start=(ct == 0),
                                stop=(ct == NCT - 1),
                            )
                    emit_rope(f, t0, w, ps, qk_tiles)

            # V in [t, d] layout: lhsT = xT tile (c, t), rhs = Wv (c, d)
            v_sb = v_pool.tile([128, NTK, HPC * D], BF16, tag="v")
            for mt in range(NTK):
                h = mt // 8
                ps = ps_mm.tile([128, HPC * D], F32, tag="vmm")
                for ct in range(NCT):
                    lc = mt * 128 - h * HT
                    nc.tensor.matmul(
                        ps[:],
                        xt_halves[h][:, ct, lc : lc + 128],
                        wv_sb[:, ct, :],
                        start=(ct == 0),
                        stop=(ct == NCT - 1),
                    )
                if with_bias_v:
                    nc.vector.tensor_add(v_sb[:, mt, :], ps[:], bv_sb[:])
                else:
                    nc.scalar.copy(v_sb[:, mt, :], ps[:])
            return qk_tiles, v_sb

        def attention(b, qk_tiles, v_sb):
            """Flash-style causal attention with one global pair pipeline
            across all (q-tile, head) units, so the scores->exp->attV
            pipeline never drains at unit boundaries.

            Z is accumulated per pair on the PE into row 0 of the pair's
            scores psum after exp drained it (WAR-ordered, no extra bank),
            then folded across pairs with tiny [1,512] DVE adds."""
            yn_h = [
                yn_pool.tile([128, T], BF16, tag=f"yn{hl}", name=f"yn{hl}")
                for hl in range(HPC)
            ]
            # flatten (j, hl, pair) stream
            stream = []
            for j in range(NTQ):
                pairs = [(2 * p, 0, 0) for p in range(2 * j)]
                pairs.append((4 * j, 0, 128))
                pairs.append((4 * j + 2, 256, 384))
                for hl in range(HPC):
                    for pi, pr in enumerate(pairs):
                        stream.append((j, hl, pi, len(pairs), pr))

            unit_state = {}
            fin_backlog = []

            def emit_finalize(u):
                yps, zacc = unit_state.pop(u)
                j, hl = u
                ysb = z_pool.tile([128, TQ], F32, tag="ysb", bufs=2)
                nc.scalar.copy(ysb[:], yps[:])
                zrec1 = z_pool.tile([1, TQ], F32, tag="zrec1", bufs=2)
                nc.vector.reciprocal_approx_fast(zrec1[:], zacc[:])
                zrecb = z_pool.tile([128, TQ], F32, tag="zrecb", bufs=2)
                nc.gpsimd.partition_broadcast(zrecb[:], zrec1[:])
                return (ysb, zrecb, hl, slice(j * TQ, (j + 1) * TQ))

            def drain_finalize(ysb, zrecb, hl, jsl):
                nc.vector.tensor_mul(yn_h[hl][:, jsl], ysb[:], zrecb[:])

            ek = {}

            def emit_pair(k):
                j, hl, pi, np_, (i0, c00, c01) = stream[k]
                qT = qk_tiles[hl]
                kT = qk_tiles[2 + hl]
                sps = ps_s.tile([128, 2 * TQ], F32, tag="s", name="sps")
                for u, c0 in ((0, c00), (1, c01)):
                    i = i0 + u
                    nc.tensor.matmul(
                        sps[:, u * TQ + c0 : (u + 1) * TQ],
                        kT[:, i * TK : (i + 1) * TK],
                        qT[:, j * TQ + c0 : (j + 1) * TQ],
                        start=True,
                        stop=True,
                    )
                e = e_pool.tile([128, 2 * TQ], BF16, tag="e")
                nc.scalar.activation(
                    e[:, c00:], sps[:, c00:], EXP, bias=0.0, scale=SCALE
                )
                ek[k] = (sps, e)

            def emit_consume(k):
                j, hl, pi, np_, (i0, c00, c01) = stream[k]
                sps, e = ek.pop(k)
                u_key = (j, hl)
                if pi == 0:
                    yps = ps_y.tile([128, TQ], F32, tag="y")
                    zacc = z_pool.tile([1, TQ], F32, tag="zacc")
                    unit_state[u_key] = (yps, zacc)
                else:
                    yps, zacc = unit_state[u_key]
                nblk = 4 * j + 4
                for u, c0 in ((0, c00), (1, c01)):
                    i = i0 + u
                    eh = e[:, u * TQ + c0 : (u + 1) * TQ]
                    if i >= 4 * j:  # diagonal block: triangle mask
                        nc.vector.tensor_mul(
                            e[:, u * TQ + c0 : u * TQ + c0 + 128],
                            e[:, u * TQ + c0 : u * TQ + c0 + 128],
                            tri_sb[:],
                        )
                    nc.tensor.matmul(
                        yps[:, c0:],
                        v_sb[:, i, hl * D : (hl + 1) * D],
                        eh,
                        start=(i == 0),
                        stop=(i == nblk - 1),
                    )
                    # pair-local Z into the dead scores psum (row 0)
                    nc.tensor.matmul(
                        sps[0:1, c0:TQ],
                        ones_sb[:],
                        eh,
                        start=(u == 0),
                        stop=(u == 1),
                    )
                if pi == 0:
                    nc.vector.tensor_copy(zacc[:], sps[0:1, 0:TQ])
                else:
                    nc.vector.tensor_add(
                        zacc[:, c00:], zacc[:, c00:], sps[0:1, c00:TQ]
                    )
                if pi == np_ - 1:
                    fin_backlog.append(emit_finalize(u_key))
                    if len(fin_backlog) > 1:
                        drain_finalize(*fin_backlog.pop(0))

            for k in range(len(stream)):
                if k >= PAIR_LOOKAHEAD:
                    emit_consume(k - PAIR_LOOKAHEAD)
                emit_pair(k)
            for k in range(len(stream) - PAIR_LOOKAHEAD, len(stream)):
                emit_consume(k)
            while fin_backlog:
                drain_finalize(*fin_backlog.pop(0))
            return yn_h

        def cproj_phase(b, yn_h):
            # wide [128,1024] psum chains on ps_s (2 per mt-tile), evacuated
            # alternately by vector/scalar.
            oq = [nc.sync, nc.gpsimd]
            for mt in range(NTK):
                osb = ob_pool.tile([128, C], BF16, tag="ob")
                for w in range(2):
                    ops = ps_s.tile([128, 2 * TQ], F32, tag="s", name="cpp")
                    for u in range(2):
                        n = 2 * w + u
                        for hl in range(HPC):
                            nc.tensor.matmul(
                                ops[:, u * TQ : (u + 1) * TQ],
                                yn_h[hl][:, mt * 128 : (mt + 1) * 128],
                                wpr_sb[:, hl, n * TQ : (n + 1) * TQ],
                                start=(hl == 0),
                                stop=(hl == HPC - 1),
                            )
                    osl = slice(2 * w * TQ, 2 * (w + 1) * TQ)
                    if w % 2 == 0:
                        nc.vector.tensor_copy(osb[:, osl], ops[:])
                    else:
                        nc.scalar.copy(osb[:, osl], ops[:])
                oq[mt % 2].dma_start(
                    out[b * T + mt * 128 : b * T + (mt + 1) * 128, :], osb[:]
                )

        for b in range(B):
            qk_tiles, v_sb = qkv_phase(b, xt_cur)
            if b + 1 < B:
                xt_cur = [load_xt_half(b + 1, 0), load_xt_half(b + 1, 1)]
            yn_h = attention(b, qk_tiles, v_sb)
            cproj_phase(b, yn_h)

    nc.compile()
    return nc


# ---- host-side sharding / unsharding ----

def _rope_cos_sin():
    inv_freq = 1.0 / (ROPE_BASE ** (np.arange(0, D, 2, dtype=np.float32) / D))
    t = np.arange(T, dtype=np.float32)
    freqs = np.outer(t, inv_freq).astype(np.float32)
    emb = np.concatenate([freqs, freqs], axis=-1)
    return np.cos(emb).astype(np.float32), np.sin(emb).astype(np.float32)


def _tri():
    a = np.arange(128)[:, None]
    c = np.arange(128)[None, :]
    return (a <= c).astype(np.float32).astype(ml_dtypes.bfloat16)


_PROGRAM_CACHE = {}


def _get_program(with_bias_qk, with_bias_v):
    key = (with_bias_qk, with_bias_v)
    if key not in _PROGRAM_CACHE:
        _PROGRAM_CACHE[key] = _build_program(with_bias_qk, with_bias_v)
    return _PROGRAM_CACHE[key]


def _make_in_maps(x, W_attn, b_attn, W_proj):
    bf = ml_dtypes.bfloat16
    x = np.asarray(x, dtype=np.float32)
    W_attn = np.asarray(W_attn, dtype=np.float32)
    b_attn = np.asarray(b_attn, dtype=np.float32)
    W_proj = np.asarray(W_proj, dtype=np.float32)

    xT = np.ascontiguousarray(
        x.transpose(2, 0, 1).reshape(C, BT)
    ).astype(bf)
    Wq, Wk, Wv = W_attn[:, :C], W_attn[:, C : 2 * C], W_attn[:, 2 * C :]
    bq, bk, bvv = b_attn[:C], b_attn[C : 2 * C], b_attn[2 * C :]
    cos, sin = _rope_cos_sin()
    cosT = np.ascontiguousarray(cos.T)
    sinNT = np.ascontiguousarray(sin.T).copy()
    sinNT[:HD, :] *= -1.0  # sign-folded for the rotate_half DVE trick
    tri = _tri()

    in_maps = []
    for c in range(N_CORES):
        h0, h1 = HPC * c, HPC * c + 1
        sl0, sl1 = slice(h0 * D, (h0 + 1) * D), slice(h1 * D, (h1 + 1) * D)
        wqk_c = np.concatenate(
            [Wq[:, sl0], Wq[:, sl1], Wk[:, sl0], Wk[:, sl1]], axis=1
        ).astype(bf).reshape(NCT, 128, 4 * D).transpose(1, 0, 2)
        wv_c = (np.concatenate([Wv[:, sl0], Wv[:, sl1]], axis=1)
                .astype(bf).reshape(NCT, 128, HPC * D).transpose(1, 0, 2))
        wpr_c = (np.concatenate([W_proj[sl0, :], W_proj[sl1, :]], axis=0)
                 .astype(bf).reshape(HPC, 128, C).transpose(1, 0, 2))
        bqk_c = np.concatenate([bq[sl0], bq[sl1], bk[sl0], bk[sl1]]).astype(
            np.float32
        ).reshape(4, 128).T
        bv_c = np.concatenate([bvv[sl0], bvv[sl1]]).astype(np.float32)
        in_maps.append(
            {
                "xT": xT,
                "wqk": np.ascontiguousarray(wqk_c),
                "wv": np.ascontiguousarray(wv_c),
                "wpr": np.ascontiguousarray(wpr_c),
                "bqk": np.ascontiguousarray(bqk_c),
                "bqkr": np.ascontiguousarray(np.concatenate([bqk_c[64:], bqk_c[:64]], axis=0)),
                "bv": bv_c,
                "cosT": cosT,
                "sinNT": sinNT,
                "tri": tri,
            }
        )
    return in_maps


def _ensure_ntff_hook():
    """Bridge the missing antenv.axon_hooks module so trace=True can profile.

    The axon boot code registers an NTFF profiling hook via
    antenv.axon_hooks, which this image's antenv package lacks. Install a
    minimal in-memory module and register the ctypes-based hook from
    trn_agent_boot. Only used for profiling runs; best-effort.
    """
    import types

    if "antenv.axon_hooks" in sys.modules:
        return
    try:
        import antenv

        mod = types.ModuleType("antenv.axon_hooks")
        holder = {"hook": None}
        mod.set_axon_ntff_profile_hook = lambda h: holder.__setitem__("hook", h)
        mod.get_axon_ntff_profile_hook = lambda: holder["hook"]
        sys.modules["antenv.axon_hooks"] = mod
        antenv.axon_hooks = mod
        axon_site = "/root/.axon_site"
        if os.path.isdir(axon_site) and axon_site not in sys.path:
            sys.path.insert(0, axon_site)
        from trn_agent_boot.trn_boot import _ntff_profile_via_ctypes

        hook = _ntff_profile_via_ctypes("/opt/axon/libaxon_pjrt.so")
        if hook is not None:
            mod.set_axon_ntff_profile_hook(hook)
    except Exception as e:  # profiling is best-effort
        print(f"[ntff hook unavailable: {type(e).__name__}: {e}]", flush=True)


def run(x, W_attn, b_attn, W_proj, b_proj, trace=False):
    if trace:
        _ensure_ntff_hook()
        import concourse.bass_utils as _bu

        _bu.upload_artifacts = lambda tmpdir: f"local://{tmpdir}"
    b_attn = np.asarray(b_attn, dtype=np.float32)
    b_proj = np.asarray(b_proj, dtype=np.float32)
    with_bias_qk = bool(np.any(b_attn[: 2 * C] != 0.0))
    with_bias_v = bool(np.any(b_attn[2 * C :] != 0.0))
    nc = _get_program(with_bias_qk, with_bias_v)
    in_maps = _make_in_maps(x, W_attn, b_attn, W_proj)
    res = run_bass_kernel_spmd(
        nc, in_maps, list(range(N_CORES)), trace=trace
    )
    acc = np.zeros((BT, C), dtype=np.float32)
    for r in res.results:
        acc += np.asarray(r["out"], dtype=np.float32)
    acc += b_proj[None, :]
    return acc.reshape(B, T, C).astype(np.float32), res


def kernel(x, W_attn, b_attn, W_proj, b_proj):
    out, _ = run(x, W_attn, b_attn, W_proj, b_proj, trace=False)
    return out


# revision 23
# speedup vs baseline: 1.0441x; 1.0441x over previous
"""Trainium2 Bass kernel for causal self-attention with RoPE (tensor-parallel over 8 cores).

Contract: kernel(**inputs) takes full unsharded inputs (x, W_attn, b_attn,
W_proj, b_proj), shards across 8 NeuronCores (2 heads each), runs one SPMD
Bass/Tile kernel, and host-reduces the partial c_proj outputs.

Design notes:
- RoPE entirely on DVE + gpsimd via partition-shifted reads of the chain
  psum with a sign-folded sin table (no rotation matmuls, no copies).
- q/k projection chains are double-wide ([128,1024] psum, 32 matmuls) so
  the 2-slot psum rotation has ~14us of slack over the rope drain.
- Causal column restriction: diagonal key-blocks only compute score/exp/
  attV/Z columns >= c0; a single shared [128,128] triangle mask handles
  the block-diagonal boundary.
- Softmax denominator Z accumulated on DVE (f32), finalized with a gpsimd
  partition_all_reduce; 1/Z fused into the y-psum evacuation.
- Heads interleaved per q-tile with double-buffered y-PSUM.
"""

import os
import sys

import numpy as np

for _p in ("/opt/trn_rl_repo",):
    if os.path.isdir(_p) and _p not in sys.path:
        sys.path.insert(0, _p)

import ml_dtypes
from contextlib import ExitStack

import concourse.bass as bass
import concourse.tile as tile
from concourse import bacc, bass_isa, mybir
from concourse.bass_utils import run_bass_kernel_spmd

# ---- problem constants (hardcoded per contract) ----
B, T, C = 2, 2048, 2048
H, D = 16, 128
N_CORES = 8
HPC = H // N_CORES  # heads per core = 2
ROPE_BASE = 10000.0
SCALE = float(1.0 / np.sqrt(D))
TQ = 512            # query tile (free dim of scores matmul)
NTQ = T // TQ       # 4
TK = 128            # key tile (partition dim of scoresT)
NTK = T // TK       # 16
NCT = C // 128      # 16 contraction tiles for projections
BT = B * T
HD = D // 2         # rope half

F32 = mybir.dt.float32
BF16 = mybir.dt.bfloat16

ADD = mybir.AluOpType.add
MULT = mybir.AluOpType.mult
EXP = mybir.ActivationFunctionType.Exp

PAIR_LOOKAHEAD = 2  # score-pairs ahead of attV in the attention pipeline


def _build_program(with_bias_qk: bool, with_bias_v: bool):
    nc = bacc.Bacc(
        "TRN2", target_bir_lowering=False, debug=False, num_devices=N_CORES
    )

    xT = nc.dram_tensor("xT", [C, BT], BF16, kind="ExternalInput").ap()
    wqk = nc.dram_tensor("wqk", [128, NCT, 4 * D], BF16, kind="ExternalInput").ap()
    wv = nc.dram_tensor("wv", [128, NCT, HPC * D], BF16, kind="ExternalInput").ap()
    wpr = nc.dram_tensor("wpr", [128, HPC, C], BF16, kind="ExternalInput").ap()
    bqk = nc.dram_tensor("bqk", [128, 4], F32, kind="ExternalInput").ap()
    bqkr = nc.dram_tensor("bqkr", [128, 4], F32, kind="ExternalInput").ap()
    bv = nc.dram_tensor("bv", [HPC * D], F32, kind="ExternalInput").ap()
    cosT = nc.dram_tensor("cosT", [D, T], F32, kind="ExternalInput").ap()
    sinNT = nc.dram_tensor("sinNT", [D, T], F32, kind="ExternalInput").ap()
    tri = nc.dram_tensor("tri", [128, 128], BF16, kind="ExternalInput").ap()
    out = nc.dram_tensor("out", [BT, C], BF16, kind="ExternalOutput").ap()

    with tile.TileContext(nc) as tc, ExitStack() as ctx:
        consts = ctx.enter_context(tc.tile_pool(name="consts", bufs=1))
        xt_pool = ctx.enter_context(tc.tile_pool(name="xt", bufs=1))
        qk_pool = ctx.enter_context(tc.tile_pool(name="qk", bufs=1))
        v_pool = ctx.enter_context(tc.tile_pool(name="v", bufs=1))
        e_pool = ctx.enter_context(tc.tile_pool(name="e", bufs=6))
        r_pool = ctx.enter_context(tc.tile_pool(name="rp", bufs=2))
        z_pool = ctx.enter_context(tc.tile_pool(name="zs", bufs=3))
        yn_pool = ctx.enter_context(tc.tile_pool(name="yn", bufs=1))
        ob_pool = ctx.enter_context(tc.tile_pool(name="ob", bufs=3))
        ps_mm = ctx.enter_context(tc.tile_pool(name="ps_mm", bufs=2, space="PSUM"))
        ps_s = ctx.enter_context(tc.tile_pool(name="ps_s", bufs=2, space="PSUM"))
        ps_y = ctx.enter_context(tc.tile_pool(name="ps_y", bufs=2, space="PSUM"))

        # ---- initial loads: wqk in 3 chunks across the 3 DMA-capable queues,
        # then x strips round-robin ----
        qs = [nc.sync, nc.gpsimd, nc.scalar]
        wqk_sb = consts.tile([128, NCT, 4 * D], BF16)
        for i, sl in enumerate((slice(0, 5), slice(5, 10), slice(10, 16))):
            qs[i].dma_start(wqk_sb[:, sl, :], wqk[:, sl, :])

        def load_xt(b, queues):
            xt_sb = xt_pool.tile([128, NCT, T], BF16, tag="xt")
            for ct in range(NCT):
                queues[ct % len(queues)].dma_start(
                    xt_sb[:, ct, :],
                    xT[ct * 128 : (ct + 1) * 128, b * T : (b + 1) * T],
                )
            return xt_sb

        xt_b0 = load_xt(0, qs)

        cos_sb = consts.tile([128, T], F32)
        nc.sync.dma_start(cos_sb[:], cosT[:])
        sin_sb = consts.tile([128, T], F32)
        nc.gpsimd.dma_start(sin_sb[:], sinNT[:])
        tri_sb = consts.tile([128, 128], BF16)
        nc.scalar.dma_start(tri_sb[:], tri[:])
        wv_sb = consts.tile([128, NCT, HPC * D], BF16)
        nc.scalar.dma_start(wv_sb[:], wv[:])
        wpr_sb = consts.tile([128, HPC, C], BF16)
        nc.sync.dma_start(wpr_sb[:], wpr[:])
        if with_bias_qk:
            bqk_sb = consts.tile([128, 4], F32)
            nc.gpsimd.dma_start(bqk_sb[:], bqk[:])
        if with_bias_v:
            bv_sb = consts.tile([128, HPC * D], F32)
            nc.gpsimd.dma_start(bv_sb[:], bv.to_broadcast((128, HPC * D)))

        def emit_rope(f, t, w, ps, qk_tiles):
            """Matmul-free rope over w cols starting at q-tile t:
            qk[f][:, tsl] = (q+b)*cos + rot_half(q+b)*sinN.
            Three DVE passes read the chain psum (shifted-base psum reads
            are legal); the final add runs on gpsimd."""
            tsl = slice(t * TQ, t * TQ + w)
            b_all = bqk_sb[:, f : f + 1] if with_bias_qk else 0.0
            b_lo = bqk_sb[0:HD, f : f + 1] if with_bias_qk else 0.0
            b_hi = bqk_sb[HD:D, f : f + 1] if with_bias_qk else 0.0
            t1 = r_pool.tile([128, 2 * TQ], BF16, tag="r1")
            nc.vector.scalar_tensor_tensor(
                t1[:, 0:w], ps[:, 0:w], b_all, cos_sb[:, tsl], op0=ADD, op1=MULT
            )
            t2 = r_pool.tile([128, 2 * TQ], BF16, tag="r2")
            nc.vector.scalar_tensor_tensor(
                t2[0:HD, 0:w], ps[HD:D, 0:w], b_hi, sin_sb[0:HD, tsl],
                op0=ADD, op1=MULT,
            )
            nc.vector.scalar_tensor_tensor(
                t2[HD:D, 0:w], ps[0:HD, 0:w], b_lo, sin_sb[HD:D, tsl],
                op0=ADD, op1=MULT,
            )
            nc.gpsimd.tensor_add(qk_tiles[f][:, tsl], t1[:, 0:w], t2[:, 0:w])

        def qkv_phase(b, xt_sb):
            """QKV projections + RoPE for batch b. Returns (qk_tiles, v_sb)."""
            # q/k feature tiles: 0=q_h0, 1=q_h1, 2=k_h0, 3=k_h1
            qk_tiles = [
                qk_pool.tile([128, T], BF16, tag=f"qk{f}", name=f"qkt{f}")
                for f in range(4)
            ]
            if b == 0:
                # cold start: t=0 for all four f-tiles ct-major so the PE
                # consumes xT strips as the initial DMAs land.
                cold_a = ps_s.tile([128, 2 * TQ], F32, tag="s", name="cold_a")
                cold_b = ps_s.tile([128, 2 * TQ], F32, tag="s", name="cold_b")
                t0_ps = [
                    cold_a[:, 0:TQ], cold_a[:, TQ : 2 * TQ],
                    cold_b[:, 0:TQ], cold_b[:, TQ : 2 * TQ],
                ]
                for ct in range(NCT):
                    for f in range(4):
                        nc.tensor.matmul(
                            t0_ps[f],
                            wqk_sb[:, ct, f * 128 : (f + 1) * 128],
                            xt_sb[:, ct, 0:TQ],
                            start=(ct == 0),
                            stop=(ct == NCT - 1),
                        )
                for f in range(4):
                    emit_rope(f, 0, TQ, t0_ps[f], qk_tiles)
                spans = [(1, 2), (3, 1)]
            else:
                spans = [(0, 2), (2, 2)]
            # remaining q/k chains: double-wide [128,1024] psums (32
            # matmuls, consecutive same-stationary) so the 2-slot rotation
            # has ~14us of slack over the DVE-side rope drain.
            for f in range(4):
                for t0, nt in spans:
                    w = nt * TQ
                    ps = ps_s.tile([128, 2 * TQ], F32, tag="s", name="qkps")
                    for ct in range(NCT):
                        for u in range(nt):
                            nc.tensor.matmul(
                                ps[:, u * TQ : (u + 1) * TQ],
                                wqk_sb[:, ct, f * 128 : (f + 1) * 128],
                                xt_sb[:, ct, (t0 + u) * TQ : (t0 + u + 1) * TQ],
                                start=(ct == 0),
                                stop=(ct == NCT - 1),
                            )
                    emit_rope(f, t0, w, ps, qk_tiles)

            # V in [t, d] layout: lhsT = xT tile (c, t), rhs = Wv (c, d)
            v_sb = v_pool.tile([128, NTK, HPC * D], BF16, tag="v")
            for mt in range(NTK):
                ps = ps_mm.tile([128, HPC * D], F32, tag="mm")
                for ct in range(NCT):
                    nc.tensor.matmul(
                        ps[:],
                        xt_sb[:, ct, mt * 128 : (mt + 1) * 128],
                        wv_sb[:, ct, :],
                        start=(ct == 0),
                        stop=(ct == NCT - 1),
                    )
                if with_bias_v:
                    nc.vector.tensor_add(v_sb[:, mt, :], ps[:], bv_sb[:])
                else:
                    nc.scalar.copy(v_sb[:, mt, :], ps[:])
            return qk_tiles, v_sb

        def attention(b, qk_tiles, v_sb):
            """Flash-style causal attention, heads interleaved per q-tile.

            Returns yn tiles ([d, T] bf16, one per head)."""
            yn_h = [
                yn_pool.tile([128, T], BF16, tag=f"yn{hl}", name=f"yn{hl}")
                for hl in range(HPC)
            ]
            fin_backlog = []

            def emit_finalize(yps, zacc, hl, jsl):
                zsum = z_pool.tile([128, TQ], F32, tag="zsum", bufs=2)
                nc.gpsimd.partition_all_reduce(
                    zsum[:], zacc[:], channels=128, reduce_op=bass_isa.ReduceOp.add
                )
                return (yps, zsum, hl, jsl)

            def drain_finalize(yps, zsum, hl, jsl):
                zrec = z_pool.tile([128, TQ], F32, tag="zrec", bufs=2)
                nc.vector.reciprocal_approx_fast(zrec[:], zsum[:])
                nc.vector.scalar_tensor_tensor(
                    yn_h[hl][:, jsl], yps[:], 0.0, zrec[:], op0=ADD, op1=MULT
                )

            for j in range(NTQ):
                jsl = slice(j * TQ, (j + 1) * TQ)
                nblk = 4 * j + 4
                # pairs of key-blocks: (i0, c0_of_i0, c0_of_i1); c0 = first
                # valid scores column (block-local) for causality.
                pairs = [(2 * p, 0, 0) for p in range(2 * j)]
                pairs.append((4 * j, 0, 128))
                pairs.append((4 * j + 2, 256, 384))
                npair = len(pairs)
                for hl in range(HPC):
                    qT = qk_tiles[hl]
                    kT = qk_tiles[2 + hl]
                    yps = ps_y.tile([128, TQ], F32, tag="y")
                    zacc = z_pool.tile([128, TQ], F32, tag="zacc")
                    e_tiles = [None] * npair

                    def emit_pair(p):
                        i0, c00, c01 = pairs[p]
                        sps = ps_s.tile([128, 2 * TQ], F32, tag="s")
                        for u, c0 in ((0, c00), (1, c01)):
                            i = i0 + u
                            nc.tensor.matmul(
                                sps[:, u * TQ + c0 : (u + 1) * TQ],
                                kT[:, i * TK : (i + 1) * TK],
                                qT[:, j * TQ + c0 : (j + 1) * TQ],
                                start=True,
                                stop=True,
                            )
                        e = e_pool.tile([128, 2 * TQ], BF16, tag="e")
                        # one exp over [c00 : 1024]; the gap columns
                        # [TQ : TQ+c01) hold garbage that is never read.
                        nc.scalar.activation(
                            e[:, c00:], sps[:, c00:], EXP, bias=0.0, scale=SCALE
                        )
                        e_tiles[p] = e

                    def emit_consume(p):
                        i0, c00, c01 = pairs[p]
                        e = e_tiles[p]
                        for u, c0 in ((0, c00), (1, c01)):
                            i = i0 + u
                            eh = e[:, u * TQ + c0 : (u + 1) * TQ]
                            if i >= 4 * j:  # diagonal block: triangle mask
                                nc.vector.tensor_mul(
                                    e[:, u * TQ + c0 : u * TQ + c0 + 128],
                                    e[:, u * TQ + c0 : u * TQ + c0 + 128],
                                    tri_sb[:],
                                )
                            if i == 0:
                                nc.vector.tensor_copy(zacc[:], e[:, 0:TQ])
                            else:
                                nc.vector.tensor_add(
                                    zacc[:, c0:], zacc[:, c0:], eh
                                )
                            nc.tensor.matmul(
                                yps[:, c0:],
                                v_sb[:, i, hl * D : (hl + 1) * D],
                                eh,
                                start=(i == 0),
                                stop=(i == nblk - 1),
                            )

                    for p in range(npair):
                        if p >= PAIR_LOOKAHEAD:
                            emit_consume(p - PAIR_LOOKAHEAD)
                        emit_pair(p)
                    for p in range(max(0, npair - PAIR_LOOKAHEAD), npair):
                        emit_consume(p)

                    fin_backlog.append(emit_finalize(yps, zacc, hl, jsl))
                    # drain the previous (j,hl)'s finalize now: its gpsimd
                    # all-reduce has had a full head-slot to complete, so the
                    # DVE queue won't stall on it.
                    if len(fin_backlog) > 1:
                        drain_finalize(*fin_backlog.pop(0))
            while fin_backlog:
                drain_finalize(*fin_backlog.pop(0))
            return yn_h

        def cproj_phase(b, yn_h):
            oq = [nc.sync, nc.gpsimd]
            for mt in range(NTK):
                osb = ob_pool.tile([128, C], BF16, tag="ob")
                for np_ in range(NTQ // 2):
                    ops = ps_s.tile([128, 2 * TQ], F32, tag="s")
                    for u in range(2):
                        n = 2 * np_ + u
                        nsl_ps = slice(u * TQ, (u + 1) * TQ)
                        for hl in range(HPC):
                            nc.tensor.matmul(
                                ops[:, nsl_ps],
                                yn_h[hl][:, mt * 128 : (mt + 1) * 128],
                                wpr_sb[:, hl, n * TQ : (n + 1) * TQ],
                                start=(hl == 0),
                                stop=(hl == HPC - 1),
                            )
                    osl = slice(2 * np_ * TQ, 2 * (np_ + 1) * TQ)
                    if np_ % 2 == 0:
                        nc.vector.tensor_copy(osb[:, osl], ops[:])
                    else:
                        nc.scalar.copy(osb[:, osl], ops[:])
                oq[mt % 2].dma_start(
                    out[b * T + mt * 128 : b * T + (mt + 1) * 128, :], osb[:]
                )

        xt_sb = xt_b0
        for b in range(B):
            qk_tiles, v_sb = qkv_phase(b, xt_sb)
            if b + 1 < B:
                xt_sb = load_xt(b + 1, qs)
            yn_h = attention(b, qk_tiles, v_sb)
            cproj_phase(b, yn_h)

    nc.compile()
    return nc


# ---- host-side sharding / unsharding ----

def _rope_cos_sin():
    inv_freq = 1.0 / (ROPE_BASE ** (np.arange(0, D, 2, dtype=np.float32) / D))
    t = np.arange(T, dtype=np.float32)
    freqs = np.outer(t, inv_freq).astype(np.float32)
    emb = np.concatenate([freqs, freqs], axis=-1)
    return np.cos(emb).astype(np.float32), np.sin(emb).astype(np.float32)


def _tri():
    a = np.arange(128)[:, None]
    c = np.arange(128)[None, :]
    return (a <= c).astype(np.float32).astype(ml_dtypes.bfloat16)


_PROGRAM_CACHE = {}


def _get_program(with_bias_qk, with_bias_v):
    key = (with_bias_qk, with_bias_v)
    if key not in _PROGRAM_CACHE:
        _PROGRAM_CACHE[key] = _build_program(with_bias_qk, with_bias_v)
    return _PROGRAM_CACHE[key]


def _make_in_maps(x, W_attn, b_attn, W_proj):
    bf = ml_dtypes.bfloat16
    x = np.asarray(x, dtype=np.float32)
    W_attn = np.asarray(W_attn, dtype=np.float32)
    b_attn = np.asarray(b_attn, dtype=np.float32)
    W_proj = np.asarray(W_proj, dtype=np.float32)

    xT = np.ascontiguousarray(
        x.transpose(2, 0, 1).reshape(C, BT)
    ).astype(bf)
    Wq, Wk, Wv = W_attn[:, :C], W_attn[:, C : 2 * C], W_attn[:, 2 * C :]
    bq, bk, bvv = b_attn[:C], b_attn[C : 2 * C], b_attn[2 * C :]
    cos, sin = _rope_cos_sin()
    cosT = np.ascontiguousarray(cos.T)
    sinNT = np.ascontiguousarray(sin.T).copy()
    sinNT[:HD, :] *= -1.0  # sign-folded for the rotate_half DVE trick
    tri = _tri()

    in_maps = []
    for c in range(N_CORES):
        h0, h1 = HPC * c, HPC * c + 1
        sl0, sl1 = slice(h0 * D, (h0 + 1) * D), slice(h1 * D, (h1 + 1) * D)
        wqk_c = np.concatenate(
            [Wq[:, sl0], Wq[:, sl1], Wk[:, sl0], Wk[:, sl1]], axis=1
        ).astype(bf).reshape(NCT, 128, 4 * D).transpose(1, 0, 2)
        wv_c = (np.concatenate([Wv[:, sl0], Wv[:, sl1]], axis=1)
                .astype(bf).reshape(NCT, 128, HPC * D).transpose(1, 0, 2))
        wpr_c = (np.concatenate([W_proj[sl0, :], W_proj[sl1, :]], axis=0)
                 .astype(bf).reshape(HPC, 128, C).transpose(1, 0, 2))
        bqk_c = np.concatenate([bq[sl0], bq[sl1], bk[sl0], bk[sl1]]).astype(
            np.float32
        ).reshape(4, 128).T
        bv_c = np.concatenate([bvv[sl0], bvv[sl1]]).astype(np.float32)
        in_maps.append(
            {
                "xT": xT,
                "wqk": np.ascontiguousarray(wqk_c),
                "wv": np.ascontiguousarray(wv_c),
                "wpr": np.ascontiguousarray(wpr_c),
                "bqk": np.ascontiguousarray(bqk_c),
                "bqkr": np.ascontiguousarray(
                    np.concatenate([bqk_c[64:], bqk_c[:64]], axis=0)
                ),
                "bv": bv_c,
                "cosT": cosT,
                "sinNT": sinNT,
                "tri": tri,
            }
        )
    return in_maps


def _ensure_ntff_hook():
    """Bridge the missing antenv.axon_hooks module so trace=True can profile.

    The axon boot code registers an NTFF profiling hook via
    antenv.axon_hooks, which this image's antenv package lacks. Install a
    minimal in-memory module and register the ctypes-based hook from
    trn_agent_boot. Only used for profiling runs; best-effort.
    """
    import types

    if "antenv.axon_hooks" in sys.modules:
        return
    try:
        import antenv

        mod = types.ModuleType("antenv.axon_hooks")
        holder = {"hook": None}
        mod.set_axon_ntff_profile_hook = lambda h: holder.__setitem__("hook", h)
        mod.get_axon_ntff_profile_hook = lambda: holder["hook"]
        sys.modules["antenv.axon_hooks"] = mod
        antenv.axon_hooks = mod
        axon_site = "/root/.axon_site"
        if os.path.isdir(axon_site) and axon_site not in sys.path:
            sys.path.insert(0, axon_site)
        from trn_agent_boot.trn_boot import _ntff_profile_via_ctypes

        hook = _ntff_profile_via_ctypes("/opt/axon/libaxon_pjrt.so")
        if hook is not None:
            mod.set_axon_ntff_profile_hook(hook)
    except Exception as e:  # profiling is best-effort
        print(f"[ntff hook unavailable: {type(e).__name__}: {e}]", flush=True)


def run(x, W_attn, b_attn, W_proj, b_proj, trace=False):
    if trace:
        _ensure_ntff_hook()
        import concourse.bass_utils as _bu

        _bu.upload_artifacts = lambda tmpdir: f"local://{tmpdir}"
    b_attn = np.asarray(b_attn, dtype=np.float32)
    b_proj = np.asarray(b_proj, dtype=np.float32)
    with_bias_qk = bool(np.any(b_attn[: 2 * C] != 0.0))
    with_bias_v = bool(np.any(b_attn[2 * C :] != 0.0))
    nc = _get_program(with_bias_qk, with_bias_v)
    in_maps = _make_in_maps(x, W_attn, b_attn, W_proj)
    res = run_bass_kernel_spmd(
        nc, in_maps, list(range(N_CORES)), trace=trace
    )
    acc = np.zeros((BT, C), dtype=np.float32)
    for r in res.results:
        acc += np.asarray(r["out"], dtype=np.float32)
    acc += b_proj[None, :]
    return acc.reshape(B, T, C).astype(np.float32), res


def kernel(x, W_attn, b_attn, W_proj, b_proj):
    out, _ = run(x, W_attn, b_attn, W_proj, b_proj, trace=False)
    return out


# revision 24
# speedup vs baseline: 1.0640x; 1.0190x over previous
"""Trainium2 Bass kernel for causal self-attention with RoPE (tensor-parallel over 8 cores).

Contract: kernel(**inputs) takes full unsharded inputs (x, W_attn, b_attn,
W_proj, b_proj), shards across 8 NeuronCores (2 heads each), runs one SPMD
Bass/Tile kernel, and host-reduces the partial c_proj outputs.

Design notes:
- RoPE entirely on DVE + gpsimd via partition-shifted reads of the chain
  psum with a sign-folded sin table (no rotation matmuls, no copies).
- q/k projection chains are double-wide ([128,1024] psum, 32 matmuls) so
  the 2-slot psum rotation has ~14us of slack over the rope drain.
- Causal column restriction: diagonal key-blocks only compute score/exp/
  attV/Z columns >= c0; a single shared [128,128] triangle mask handles
  the block-diagonal boundary.
- Softmax denominator Z accumulated on DVE (f32), finalized with a gpsimd
  partition_all_reduce; 1/Z fused into the y-psum evacuation.
- Heads interleaved per q-tile with double-buffered y-PSUM.
"""

import os
import sys

import numpy as np

for _p in ("/opt/trn_rl_repo",):
    if os.path.isdir(_p) and _p not in sys.path:
        sys.path.insert(0, _p)

import ml_dtypes
from contextlib import ExitStack

import concourse.bass as bass
import concourse.tile as tile
from concourse import bacc, bass_isa, mybir
from concourse.bass_utils import run_bass_kernel_spmd

# ---- problem constants (hardcoded per contract) ----
B, T, C = 2, 2048, 2048
H, D = 16, 128
N_CORES = 8
HPC = H // N_CORES  # heads per core = 2
ROPE_BASE = 10000.0
SCALE = float(1.0 / np.sqrt(D))
TQ = 512            # query tile (free dim of scores matmul)
NTQ = T // TQ       # 4
TK = 128            # key tile (partition dim of scoresT)
NTK = T // TK       # 16
NCT = C // 128      # 16 contraction tiles for projections
BT = B * T
HD = D // 2         # rope half

F32 = mybir.dt.float32
BF16 = mybir.dt.bfloat16

ADD = mybir.AluOpType.add
MULT = mybir.AluOpType.mult
EXP = mybir.ActivationFunctionType.Exp

PAIR_LOOKAHEAD = 2  # score-pairs ahead of attV in the attention pipeline


def _build_program(with_bias_qk: bool, with_bias_v: bool):
    nc = bacc.Bacc(
        "TRN2", target_bir_lowering=False, debug=False, num_devices=N_CORES
    )

    xT = nc.dram_tensor("xT", [C, BT], BF16, kind="ExternalInput").ap()
    wqk = nc.dram_tensor("wqk", [128, NCT, 4 * D], BF16, kind="ExternalInput").ap()
    wv = nc.dram_tensor("wv", [128, NCT, HPC * D], BF16, kind="ExternalInput").ap()
    wpr = nc.dram_tensor("wpr", [128, HPC, C], BF16, kind="ExternalInput").ap()
    bqk = nc.dram_tensor("bqk", [128, 4], F32, kind="ExternalInput").ap()
    bqkr = nc.dram_tensor("bqkr", [128, 4], F32, kind="ExternalInput").ap()
    bv = nc.dram_tensor("bv", [HPC * D], F32, kind="ExternalInput").ap()
    cosT = nc.dram_tensor("cosT", [D, T], F32, kind="ExternalInput").ap()
    sinNT = nc.dram_tensor("sinNT", [D, T], F32, kind="ExternalInput").ap()
    tri = nc.dram_tensor("tri", [128, 128], BF16, kind="ExternalInput").ap()
    out = nc.dram_tensor("out", [BT, C], BF16, kind="ExternalOutput").ap()

    with tile.TileContext(nc) as tc, ExitStack() as ctx:
        consts = ctx.enter_context(tc.tile_pool(name="consts", bufs=1))
        xt_pool = ctx.enter_context(tc.tile_pool(name="xt", bufs=1))
        qk_pool = ctx.enter_context(tc.tile_pool(name="qk", bufs=1))
        v_pool = ctx.enter_context(tc.tile_pool(name="v", bufs=1))
        e_pool = ctx.enter_context(tc.tile_pool(name="e", bufs=6))
        r_pool = ctx.enter_context(tc.tile_pool(name="rp", bufs=2))
        z_pool = ctx.enter_context(tc.tile_pool(name="zs", bufs=3))
        yn_pool = ctx.enter_context(tc.tile_pool(name="yn", bufs=1))
        ob_pool = ctx.enter_context(tc.tile_pool(name="ob", bufs=3))
        ps_mm = ctx.enter_context(tc.tile_pool(name="ps_mm", bufs=2, space="PSUM"))
        ps_s = ctx.enter_context(tc.tile_pool(name="ps_s", bufs=2, space="PSUM"))
        ps_y = ctx.enter_context(tc.tile_pool(name="ps_y", bufs=2, space="PSUM"))

        # ---- initial loads: wqk in 3 chunks across the 3 DMA-capable queues,
        # then x strips round-robin ----
        qs = [nc.sync, nc.gpsimd, nc.scalar]
        wqk_sb = consts.tile([128, NCT, 4 * D], BF16)
        for i, sl in enumerate((slice(0, 5), slice(5, 10), slice(10, 16))):
            qs[i].dma_start(wqk_sb[:, sl, :], wqk[:, sl, :])

        def load_xt(b, queues):
            xt_sb = xt_pool.tile([128, NCT, T], BF16, tag="xt")
            for ct in range(NCT):
                queues[ct % len(queues)].dma_start(
                    xt_sb[:, ct, :],
                    xT[ct * 128 : (ct + 1) * 128, b * T : (b + 1) * T],
                )
            return xt_sb

        xt_b0 = load_xt(0, qs)

        cos_sb = consts.tile([128, T], F32)
        nc.sync.dma_start(cos_sb[:], cosT[:])
        sin_sb = consts.tile([128, T], F32)
        nc.gpsimd.dma_start(sin_sb[:], sinNT[:])
        tri_sb = consts.tile([128, 128], BF16)
        nc.scalar.dma_start(tri_sb[:], tri[:])
        wv_sb = consts.tile([128, NCT, HPC * D], BF16)
        nc.scalar.dma_start(wv_sb[:], wv[:])
        wpr_sb = consts.tile([128, HPC, C], BF16)
        nc.sync.dma_start(wpr_sb[:], wpr[:])
        if with_bias_qk:
            bqk_sb = consts.tile([128, 4], F32)
            nc.gpsimd.dma_start(bqk_sb[:], bqk[:])
        if with_bias_v:
            bv_sb = consts.tile([128, HPC * D], F32)
            nc.gpsimd.dma_start(bv_sb[:], bv.to_broadcast((128, HPC * D)))

        def emit_rope(f, t, w, ps, qk_tiles):
            """Matmul-free rope over w cols starting at q-tile t:
            qk[f][:, tsl] = (q+b)*cos + rot_half(q+b)*sinN.
            Three DVE passes read the chain psum (shifted-base psum reads
            are legal); the final add runs on gpsimd."""
            tsl = slice(t * TQ, t * TQ + w)
            b_all = bqk_sb[:, f : f + 1] if with_bias_qk else 0.0
            b_lo = bqk_sb[0:HD, f : f + 1] if with_bias_qk else 0.0
            b_hi = bqk_sb[HD:D, f : f + 1] if with_bias_qk else 0.0
            t1 = r_pool.tile([128, 2 * TQ], F32, tag="r1")
            nc.vector.scalar_tensor_tensor(
                t1[:, 0:w], ps[:, 0:w], b_all, cos_sb[:, tsl], op0=ADD, op1=MULT
            )
            t2 = r_pool.tile([128, 2 * TQ], F32, tag="r2")
            nc.vector.scalar_tensor_tensor(
                t2[0:HD, 0:w], ps[HD:D, 0:w], b_hi, sin_sb[0:HD, tsl],
                op0=ADD, op1=MULT,
            )
            nc.vector.scalar_tensor_tensor(
                t2[HD:D, 0:w], ps[0:HD, 0:w], b_lo, sin_sb[HD:D, tsl],
                op0=ADD, op1=MULT,
            )
            nc.vector.tensor_add(qk_tiles[f][:, tsl], t1[:, 0:w], t2[:, 0:w])

        def qkv_phase(b, xt_sb):
            """QKV projections + RoPE for batch b. Returns (qk_tiles, v_sb)."""
            # q/k feature tiles: 0=q_h0, 1=q_h1, 2=k_h0, 3=k_h1
            qk_tiles = [
                qk_pool.tile([128, T], BF16, tag=f"qk{f}", name=f"qkt{f}")
                for f in range(4)
            ]
            if b == 0:
                # cold start: t=0 for all four f-tiles ct-major so the PE
                # consumes xT strips as the initial DMAs land.
                cold_a = ps_s.tile([128, 2 * TQ], F32, tag="s", name="cold_a")
                cold_b = ps_s.tile([128, 2 * TQ], F32, tag="s", name="cold_b")
                t0_ps = [
                    cold_a[:, 0:TQ], cold_a[:, TQ : 2 * TQ],
                    cold_b[:, 0:TQ], cold_b[:, TQ : 2 * TQ],
                ]
                for ct in range(NCT):
                    for f in range(4):
                        nc.tensor.matmul(
                            t0_ps[f],
                            wqk_sb[:, ct, f * 128 : (f + 1) * 128],
                            xt_sb[:, ct, 0:TQ],
                            start=(ct == 0),
                            stop=(ct == NCT - 1),
                        )
                for f in range(4):
                    emit_rope(f, 0, TQ, t0_ps[f], qk_tiles)
                spans = [(1, 2), (3, 1)]
            else:
                spans = [(0, 2), (2, 2)]
            # remaining q/k chains: double-wide [128,1024] psums (32
            # matmuls, consecutive same-stationary) so the 2-slot rotation
            # has ~14us of slack over the DVE-side rope drain.
            for f in range(4):
                for t0, nt in spans:
                    w = nt * TQ
                    ps = ps_s.tile([128, 2 * TQ], F32, tag="s", name="qkps")
                    for ct in range(NCT):
                        for u in range(nt):
                            nc.tensor.matmul(
                                ps[:, u * TQ : (u + 1) * TQ],
                                wqk_sb[:, ct, f * 128 : (f + 1) * 128],
                                xt_sb[:, ct, (t0 + u) * TQ : (t0 + u + 1) * TQ],
                                start=(ct == 0),
                                stop=(ct == NCT - 1),
                            )
                    emit_rope(f, t0, w, ps, qk_tiles)

            # V in [t, d] layout: lhsT = xT tile (c, t), rhs = Wv (c, d)
            v_sb = v_pool.tile([128, NTK, HPC * D], BF16, tag="v")
            for mt in range(NTK):
                ps = ps_mm.tile([128, HPC * D], F32, tag="mm")
                for ct in range(NCT):
                    nc.tensor.matmul(
                        ps[:],
                        xt_sb[:, ct, mt * 128 : (mt + 1) * 128],
                        wv_sb[:, ct, :],
                        start=(ct == 0),
                        stop=(ct == NCT - 1),
                    )
                if with_bias_v:
                    nc.vector.tensor_add(v_sb[:, mt, :], ps[:], bv_sb[:])
                else:
                    nc.scalar.copy(v_sb[:, mt, :], ps[:])
            return qk_tiles, v_sb

        def attention(b, qk_tiles, v_sb):
            """Flash-style causal attention, heads interleaved per q-tile.

            Returns yn tiles ([d, T] bf16, one per head)."""
            yn_h = [
                yn_pool.tile([128, T], BF16, tag=f"yn{hl}", name=f"yn{hl}")
                for hl in range(HPC)
            ]
            fin_backlog = []

            def emit_finalize(yps, zacc, hl, jsl):
                zsum = z_pool.tile([128, TQ], F32, tag="zsum", bufs=2)
                nc.gpsimd.partition_all_reduce(
                    zsum[:], zacc[:], channels=128, reduce_op=bass_isa.ReduceOp.add
                )
                return (yps, zsum, hl, jsl)

            def drain_finalize(yps, zsum, hl, jsl):
                zrec = z_pool.tile([128, TQ], F32, tag="zrec", bufs=2)
                nc.vector.reciprocal_approx_fast(zrec[:], zsum[:])
                nc.vector.scalar_tensor_tensor(
                    yn_h[hl][:, jsl], yps[:], 0.0, zrec[:], op0=ADD, op1=MULT
                )

            for j in range(NTQ):
                jsl = slice(j * TQ, (j + 1) * TQ)
                nblk = 4 * j + 4
                # pairs of key-blocks: (i0, c0_of_i0, c0_of_i1); c0 = first
                # valid scores column (block-local) for causality.
                pairs = [(2 * p, 0, 0) for p in range(2 * j)]
                pairs.append((4 * j, 0, 128))
                pairs.append((4 * j + 2, 256, 384))
                npair = len(pairs)
                for hl in range(HPC):
                    qT = qk_tiles[hl]
                    kT = qk_tiles[2 + hl]
                    yps = ps_y.tile([128, TQ], F32, tag="y")
                    zacc = z_pool.tile([128, TQ], F32, tag="zacc")
                    e_tiles = [None] * npair

                    def emit_pair(p):
                        i0, c00, c01 = pairs[p]
                        sps = ps_s.tile([128, 2 * TQ], F32, tag="s")
                        for u, c0 in ((0, c00), (1, c01)):
                            i = i0 + u
                            nc.tensor.matmul(
                                sps[:, u * TQ + c0 : (u + 1) * TQ],
                                kT[:, i * TK : (i + 1) * TK],
                                qT[:, j * TQ + c0 : (j + 1) * TQ],
                                start=True,
                                stop=True,
                            )
                        e = e_pool.tile([128, 2 * TQ], BF16, tag="e")
                        # one exp over [c00 : 1024]; the gap columns
                        # [TQ : TQ+c01) hold garbage that is never read.
                        nc.scalar.activation(
                            e[:, c00:], sps[:, c00:], EXP, bias=0.0, scale=SCALE
                        )
                        e_tiles[p] = e

                    def emit_consume(p):
                        i0, c00, c01 = pairs[p]
                        e = e_tiles[p]
                        for u, c0 in ((0, c00), (1, c01)):
                            i = i0 + u
                            eh = e[:, u * TQ + c0 : (u + 1) * TQ]
                            if i >= 4 * j:  # diagonal block: triangle mask
                                nc.vector.tensor_mul(
                                    e[:, u * TQ + c0 : u * TQ + c0 + 128],
                                    e[:, u * TQ + c0 : u * TQ + c0 + 128],
                                    tri_sb[:],
                                )
                            if i == 0:
                                nc.vector.tensor_copy(zacc[:], e[:, 0:TQ])
                            else:
                                nc.vector.tensor_add(
                                    zacc[:, c0:], zacc[:, c0:], eh
                                )
                            nc.tensor.matmul(
                                yps[:, c0:],
                                v_sb[:, i, hl * D : (hl + 1) * D],
                                eh,
                                start=(i == 0),
                                stop=(i == nblk - 1),
                            )

                    for p in range(npair):
                        if p >= PAIR_LOOKAHEAD:
                            emit_consume(p - PAIR_LOOKAHEAD)
                        emit_pair(p)
                    for p in range(max(0, npair - PAIR_LOOKAHEAD), npair):
                        emit_consume(p)

                    fin_backlog.append(emit_finalize(yps, zacc, hl, jsl))
                    # drain the previous (j,hl)'s finalize now: its gpsimd
                    # all-reduce has had a full head-slot to complete, so the
                    # DVE queue won't stall on it.
                    if len(fin_backlog) > 1:
                        drain_finalize(*fin_backlog.pop(0))
            while fin_backlog:
                drain_finalize(*fin_backlog.pop(0))
            return yn_h

        def cproj_phase(b, yn_h):
            oq = [nc.sync, nc.gpsimd]
            for mt in range(NTK):
                osb = ob_pool.tile([128, C], BF16, tag="ob")
                for np_ in range(NTQ // 2):
                    ops = ps_s.tile([128, 2 * TQ], F32, tag="s")
                    for u in range(2):
                        n = 2 * np_ + u
                        nsl_ps = slice(u * TQ, (u + 1) * TQ)
                        for hl in range(HPC):
                            nc.tensor.matmul(
                                ops[:, nsl_ps],
                                yn_h[hl][:, mt * 128 : (mt + 1) * 128],
                                wpr_sb[:, hl, n * TQ : (n + 1) * TQ],
                                start=(hl == 0),
                                stop=(hl == HPC - 1),
                            )
                    osl = slice(2 * np_ * TQ, 2 * (np_ + 1) * TQ)
                    if np_ % 2 == 0:
                        nc.vector.tensor_copy(osb[:, osl], ops[:])
                    else:
                        nc.scalar.copy(osb[:, osl], ops[:])
                oq[mt % 2].dma_start(
                    out[b * T + mt * 128 : b * T + (mt + 1) * 128, :], osb[:]
                )

        xt_sb = xt_b0
        for b in range(B):
            qk_tiles, v_sb = qkv_phase(b, xt_sb)
            if b + 1 < B:
                xt_sb = load_xt(b + 1, qs)
            yn_h = attention(b, qk_tiles, v_sb)
            cproj_phase(b, yn_h)

    nc.compile()
    return nc


# ---- host-side sharding / unsharding ----

def _rope_cos_sin():
    inv_freq = 1.0 / (ROPE_BASE ** (np.arange(0, D, 2, dtype=np.float32) / D))
    t = np.arange(T, dtype=np.float32)
    freqs = np.outer(t, inv_freq).astype(np.float32)
    emb = np.concatenate([freqs, freqs], axis=-1)
    return np.cos(emb).astype(np.float32), np.sin(emb).astype(np.float32)


def _tri():
    a = np.arange(128)[:, None]
    c = np.arange(128)[None, :]
    return (a <= c).astype(np.float32).astype(ml_dtypes.bfloat16)


_PROGRAM_CACHE = {}


def _get_program(with_bias_qk, with_bias_v):
    key = (with_bias_qk, with_bias_v)
    if key not in _PROGRAM_CACHE:
        _PROGRAM_CACHE[key] = _build_program(with_bias_qk, with_bias_v)
    return _PROGRAM_CACHE[key]


def _make_in_maps(x, W_attn, b_attn, W_proj):
    bf = ml_dtypes.bfloat16
    x = np.asarray(x, dtype=np.float32)
    W_attn = np.asarray(W_attn, dtype=np.float32)
    b_attn = np.asarray(b_attn, dtype=np.float32)
    W_proj = np.asarray(W_proj, dtype=np.float32)

    xT = np.ascontiguousarray(
        x.transpose(2, 0, 1).reshape(C, BT)
    ).astype(bf)
    Wq, Wk, Wv = W_attn[:, :C], W_attn[:, C : 2 * C], W_attn[:, 2 * C :]
    bq, bk, bvv = b_attn[:C], b_attn[C : 2 * C], b_attn[2 * C :]
    cos, sin = _rope_cos_sin()
    cosT = np.ascontiguousarray(cos.T)
    sinNT = np.ascontiguousarray(sin.T).copy()
    sinNT[:HD, :] *= -1.0  # sign-folded for the rotate_half DVE trick
    tri = _tri()

    in_maps = []
    for c in range(N_CORES):
        h0, h1 = HPC * c, HPC * c + 1
        sl0, sl1 = slice(h0 * D, (h0 + 1) * D), slice(h1 * D, (h1 + 1) * D)
        wqk_c = np.concatenate(
            [Wq[:, sl0], Wq[:, sl1], Wk[:, sl0], Wk[:, sl1]], axis=1
        ).astype(bf).reshape(NCT, 128, 4 * D).transpose(1, 0, 2)
        wv_c = (np.concatenate([Wv[:, sl0], Wv[:, sl1]], axis=1)
                .astype(bf).reshape(NCT, 128, HPC * D).transpose(1, 0, 2))
        wpr_c = (np.concatenate([W_proj[sl0, :], W_proj[sl1, :]], axis=0)
                 .astype(bf).reshape(HPC, 128, C).transpose(1, 0, 2))
        bqk_c = np.concatenate([bq[sl0], bq[sl1], bk[sl0], bk[sl1]]).astype(
            np.float32
        ).reshape(4, 128).T
        bv_c = np.concatenate([bvv[sl0], bvv[sl1]]).astype(np.float32)
        in_maps.append(
            {
                "xT": xT,
                "wqk": np.ascontiguousarray(wqk_c),
                "wv": np.ascontiguousarray(wv_c),
                "wpr": np.ascontiguousarray(wpr_c),
                "bqk": np.ascontiguousarray(bqk_c),
                "bqkr": np.ascontiguousarray(
                    np.concatenate([bqk_c[64:], bqk_c[:64]], axis=0)
                ),
                "bv": bv_c,
                "cosT": cosT,
                "sinNT": sinNT,
                "tri": tri,
            }
        )
    return in_maps


def _ensure_ntff_hook():
    """Bridge the missing antenv.axon_hooks module so trace=True can profile.

    The axon boot code registers an NTFF profiling hook via
    antenv.axon_hooks, which this image's antenv package lacks. Install a
    minimal in-memory module and register the ctypes-based hook from
    trn_agent_boot. Only used for profiling runs; best-effort.
    """
    import types

    if "antenv.axon_hooks" in sys.modules:
        return
    try:
        import antenv

        mod = types.ModuleType("antenv.axon_hooks")
        holder = {"hook": None}
        mod.set_axon_ntff_profile_hook = lambda h: holder.__setitem__("hook", h)
        mod.get_axon_ntff_profile_hook = lambda: holder["hook"]
        sys.modules["antenv.axon_hooks"] = mod
        antenv.axon_hooks = mod
        axon_site = "/root/.axon_site"
        if os.path.isdir(axon_site) and axon_site not in sys.path:
            sys.path.insert(0, axon_site)
        from trn_agent_boot.trn_boot import _ntff_profile_via_ctypes

        hook = _ntff_profile_via_ctypes("/opt/axon/libaxon_pjrt.so")
        if hook is not None:
            mod.set_axon_ntff_profile_hook(hook)
    except Exception as e:  # profiling is best-effort
        print(f"[ntff hook unavailable: {type(e).__name__}: {e}]", flush=True)


def run(x, W_attn, b_attn, W_proj, b_proj, trace=False):
    if trace:
        _ensure_ntff_hook()
        import concourse.bass_utils as _bu

        _bu.upload_artifacts = lambda tmpdir: f"local://{tmpdir}"
    b_attn = np.asarray(b_attn, dtype=np.float32)
    b_proj = np.asarray(b_proj, dtype=np.float32)
    with_bias_qk = bool(np.any(b_attn[: 2 * C] != 0.0))
    with_bias_v = bool(np.any(b_attn[2 * C :] != 0.0))
    nc = _get_program(with_bias_qk, with_bias_v)
    in_maps = _make_in_maps(x, W_attn, b_attn, W_proj)
    res = run_bass_kernel_spmd(
        nc, in_maps, list(range(N_CORES)), trace=trace
    )
    acc = np.zeros((BT, C), dtype=np.float32)
    for r in res.results:
        acc += np.asarray(r["out"], dtype=np.float32)
    acc += b_proj[None, :]
    return acc.reshape(B, T, C).astype(np.float32), res


def kernel(x, W_attn, b_attn, W_proj, b_proj):
    out, _ = run(x, W_attn, b_attn, W_proj, b_proj, trace=False)
    return out


# revision 25
# speedup vs baseline: 1.0887x; 1.0232x over previous
"""Trainium2 Bass kernel for causal self-attention with RoPE (tensor-parallel over 8 cores).

Contract: kernel(**inputs) takes full unsharded inputs (x, W_attn, b_attn,
W_proj, b_proj), shards across 8 NeuronCores (2 heads each), runs one SPMD
Bass/Tile kernel, and host-reduces the partial c_proj outputs.

Design notes:
- RoPE entirely on DVE + gpsimd via partition-shifted reads of the chain
  psum with a sign-folded sin table (no rotation matmuls, no copies).
- q/k projection chains are double-wide ([128,1024] psum, 32 matmuls) so
  the 2-slot psum rotation has ~14us of slack over the rope drain.
- Causal column restriction: diagonal key-blocks only compute score/exp/
  attV/Z columns >= c0; a single shared [128,128] triangle mask handles
  the block-diagonal boundary.
- Softmax denominator Z accumulated on DVE (f32), finalized with a gpsimd
  partition_all_reduce; 1/Z fused into the y-psum evacuation.
- Heads interleaved per q-tile with double-buffered y-PSUM.
"""

import os
import sys

import numpy as np

for _p in ("/opt/trn_rl_repo",):
    if os.path.isdir(_p) and _p not in sys.path:
        sys.path.insert(0, _p)

import ml_dtypes
from contextlib import ExitStack

import concourse.bass as bass
import concourse.tile as tile
from concourse import bacc, bass_isa, mybir
from concourse.bass_utils import run_bass_kernel_spmd

# ---- problem constants (hardcoded per contract) ----
B, T, C = 2, 2048, 2048
H, D = 16, 128
N_CORES = 8
HPC = H // N_CORES  # heads per core = 2
ROPE_BASE = 10000.0
SCALE = float(1.0 / np.sqrt(D))
TQ = 512            # query tile (free dim of scores matmul)
NTQ = T // TQ       # 4
TK = 128            # key tile (partition dim of scoresT)
NTK = T // TK       # 16
NCT = C // 128      # 16 contraction tiles for projections
BT = B * T
HD = D // 2         # rope half

F32 = mybir.dt.float32
BF16 = mybir.dt.bfloat16

ADD = mybir.AluOpType.add
MULT = mybir.AluOpType.mult
EXP = mybir.ActivationFunctionType.Exp

PAIR_LOOKAHEAD = 2  # score-pairs ahead of attV in the attention pipeline


def _build_program(with_bias_qk: bool, with_bias_v: bool):
    nc = bacc.Bacc(
        "TRN2", target_bir_lowering=False, debug=False, num_devices=N_CORES
    )

    xT = nc.dram_tensor("xT", [C, BT], BF16, kind="ExternalInput").ap()
    wqk = nc.dram_tensor("wqk", [128, NCT, 4 * D], BF16, kind="ExternalInput").ap()
    wv = nc.dram_tensor("wv", [128, NCT, HPC * D], BF16, kind="ExternalInput").ap()
    wpr = nc.dram_tensor("wpr", [128, HPC, C], BF16, kind="ExternalInput").ap()
    bqk = nc.dram_tensor("bqk", [128, 4], F32, kind="ExternalInput").ap()
    bqkr = nc.dram_tensor("bqkr", [128, 4], F32, kind="ExternalInput").ap()
    bv = nc.dram_tensor("bv", [HPC * D], F32, kind="ExternalInput").ap()
    cosT = nc.dram_tensor("cosT", [D, T], F32, kind="ExternalInput").ap()
    sinNT = nc.dram_tensor("sinNT", [D, T], F32, kind="ExternalInput").ap()
    tri = nc.dram_tensor("tri", [128, 128], BF16, kind="ExternalInput").ap()
    out = nc.dram_tensor("out", [BT, C], BF16, kind="ExternalOutput").ap()

    with tile.TileContext(nc) as tc, ExitStack() as ctx:
        consts = ctx.enter_context(tc.tile_pool(name="consts", bufs=1))
        xt_pool = ctx.enter_context(tc.tile_pool(name="xt", bufs=1))
        qk_pool = ctx.enter_context(tc.tile_pool(name="qk", bufs=1))
        v_pool = ctx.enter_context(tc.tile_pool(name="v", bufs=1))
        e_pool = ctx.enter_context(tc.tile_pool(name="e", bufs=6))
        r_pool = ctx.enter_context(tc.tile_pool(name="rp", bufs=2))
        z_pool = ctx.enter_context(tc.tile_pool(name="zs", bufs=3))
        yn_pool = ctx.enter_context(tc.tile_pool(name="yn", bufs=1))
        ob_pool = ctx.enter_context(tc.tile_pool(name="ob", bufs=3))
        ps_mm = ctx.enter_context(tc.tile_pool(name="ps_mm", bufs=2, space="PSUM"))
        ps_s = ctx.enter_context(tc.tile_pool(name="ps_s", bufs=2, space="PSUM"))
        ps_y = ctx.enter_context(tc.tile_pool(name="ps_y", bufs=2, space="PSUM"))

        # ---- initial loads: wqk in 3 chunks across the 3 DMA-capable queues,
        # then x strips round-robin ----
        qs = [nc.sync, nc.gpsimd, nc.scalar]
        wqk_sb = consts.tile([128, NCT, 4 * D], BF16)
        for i, sl in enumerate((slice(0, 5), slice(5, 10), slice(10, 16))):
            qs[i].dma_start(wqk_sb[:, sl, :], wqk[:, sl, :])

        def load_xt(b, queues):
            xt_sb = xt_pool.tile([128, NCT, T], BF16, tag="xt")
            for ct in range(NCT):
                queues[ct % len(queues)].dma_start(
                    xt_sb[:, ct, :],
                    xT[ct * 128 : (ct + 1) * 128, b * T : (b + 1) * T],
                )
            return xt_sb

        xt_b0 = load_xt(0, qs)

        cos_sb = consts.tile([128, T], F32)
        nc.sync.dma_start(cos_sb[:], cosT[:])
        sin_sb = consts.tile([128, T], F32)
        nc.gpsimd.dma_start(sin_sb[:], sinNT[:])
        tri_sb = consts.tile([128, 128], BF16)
        nc.scalar.dma_start(tri_sb[:], tri[:])
        wv_sb = consts.tile([128, NCT, HPC * D], BF16)
        nc.scalar.dma_start(wv_sb[:], wv[:])
        wpr_sb = consts.tile([128, HPC, C], BF16)
        nc.sync.dma_start(wpr_sb[:], wpr[:])
        if with_bias_qk:
            bqk_sb = consts.tile([128, 4], F32)
            nc.gpsimd.dma_start(bqk_sb[:], bqk[:])
        if with_bias_v:
            bv_sb = consts.tile([128, HPC * D], F32)
            nc.gpsimd.dma_start(bv_sb[:], bv.to_broadcast((128, HPC * D)))

        def emit_rope(f, t, w, ps, qk_tiles):
            """Matmul-free rope over w cols starting at q-tile t:
            qk[f][:, tsl] = (q+b)*cos + rot_half(q+b)*sinN.
            Three DVE passes read the chain psum (shifted-base psum reads
            are legal); the final add runs on gpsimd."""
            tsl = slice(t * TQ, t * TQ + w)
            b_all = bqk_sb[:, f : f + 1] if with_bias_qk else 0.0
            b_lo = bqk_sb[0:HD, f : f + 1] if with_bias_qk else 0.0
            b_hi = bqk_sb[HD:D, f : f + 1] if with_bias_qk else 0.0
            t1 = r_pool.tile([128, 2 * TQ], F32, tag="r1")
            nc.vector.scalar_tensor_tensor(
                t1[:, 0:w], ps[:, 0:w], b_all, cos_sb[:, tsl], op0=ADD, op1=MULT
            )
            t2 = r_pool.tile([128, 2 * TQ], F32, tag="r2")
            nc.vector.scalar_tensor_tensor(
                t2[0:HD, 0:w], ps[HD:D, 0:w], b_hi, sin_sb[0:HD, tsl],
                op0=ADD, op1=MULT,
            )
            nc.vector.scalar_tensor_tensor(
                t2[HD:D, 0:w], ps[0:HD, 0:w], b_lo, sin_sb[HD:D, tsl],
                op0=ADD, op1=MULT,
            )
            nc.vector.tensor_add(qk_tiles[f][:, tsl], t1[:, 0:w], t2[:, 0:w])

        def qkv_phase(b, xt_sb):
            """QKV projections + RoPE for batch b. Returns (qk_tiles, v_sb)."""
            # q/k feature tiles: 0=q_h0, 1=q_h1, 2=k_h0, 3=k_h1
            qk_tiles = [
                qk_pool.tile([128, T], BF16, tag=f"qk{f}", name=f"qkt{f}")
                for f in range(4)
            ]
            if b == 0:
                # cold start: t=0 for all four f-tiles ct-major so the PE
                # consumes xT strips as the initial DMAs land.
                cold_a = ps_s.tile([128, 2 * TQ], F32, tag="s", name="cold_a")
                cold_b = ps_s.tile([128, 2 * TQ], F32, tag="s", name="cold_b")
                t0_ps = [
                    cold_a[:, 0:TQ], cold_a[:, TQ : 2 * TQ],
                    cold_b[:, 0:TQ], cold_b[:, TQ : 2 * TQ],
                ]
                for ct in range(NCT):
                    for f in range(4):
                        nc.tensor.matmul(
                            t0_ps[f],
                            wqk_sb[:, ct, f * 128 : (f + 1) * 128],
                            xt_sb[:, ct, 0:TQ],
                            start=(ct == 0),
                            stop=(ct == NCT - 1),
                        )
                for f in range(4):
                    emit_rope(f, 0, TQ, t0_ps[f], qk_tiles)
            for f in range(4):
                for t in range(NTQ):
                    if b == 0 and t == 0:
                        continue
                    ps = ps_mm.tile([128, TQ], F32, tag="mm")
                    for ct in range(NCT):
                        nc.tensor.matmul(
                            ps[:],
                            wqk_sb[:, ct, f * 128 : (f + 1) * 128],
                            xt_sb[:, ct, t * TQ : (t + 1) * TQ],
                            start=(ct == 0),
                            stop=(ct == NCT - 1),
                        )
                    emit_rope(f, t, TQ, ps, qk_tiles)

            # V in [t, d] layout: lhsT = xT tile (c, t), rhs = Wv (c, d)
            v_sb = v_pool.tile([128, NTK, HPC * D], BF16, tag="v")
            for mt in range(NTK):
                ps = ps_mm.tile([128, HPC * D], F32, tag="mm")
                for ct in range(NCT):
                    nc.tensor.matmul(
                        ps[:],
                        xt_sb[:, ct, mt * 128 : (mt + 1) * 128],
                        wv_sb[:, ct, :],
                        start=(ct == 0),
                        stop=(ct == NCT - 1),
                    )
                if with_bias_v:
                    nc.vector.tensor_add(v_sb[:, mt, :], ps[:], bv_sb[:])
                else:
                    nc.scalar.copy(v_sb[:, mt, :], ps[:])
            return qk_tiles, v_sb

        def attention(b, qk_tiles, v_sb):
            """Flash-style causal attention, heads interleaved per q-tile.

            Returns yn tiles ([d, T] bf16, one per head)."""
            yn_h = [
                yn_pool.tile([128, T], BF16, tag=f"yn{hl}", name=f"yn{hl}")
                for hl in range(HPC)
            ]
            fin_backlog = []

            def emit_finalize(yps, zacc, hl, jsl):
                zsum = z_pool.tile([128, TQ], F32, tag="zsum", bufs=2)
                nc.gpsimd.partition_all_reduce(
                    zsum[:], zacc[:], channels=128, reduce_op=bass_isa.ReduceOp.add
                )
                return (yps, zsum, hl, jsl)

            def drain_finalize(yps, zsum, hl, jsl):
                zrec = z_pool.tile([128, TQ], F32, tag="zrec", bufs=2)
                nc.vector.reciprocal_approx_fast(zrec[:], zsum[:])
                nc.vector.scalar_tensor_tensor(
                    yn_h[hl][:, jsl], yps[:], 0.0, zrec[:], op0=ADD, op1=MULT
                )

            for j in range(NTQ):
                jsl = slice(j * TQ, (j + 1) * TQ)
                nblk = 4 * j + 4
                # pairs of key-blocks: (i0, c0_of_i0, c0_of_i1); c0 = first
                # valid scores column (block-local) for causality.
                pairs = [(2 * p, 0, 0) for p in range(2 * j)]
                pairs.append((4 * j, 0, 128))
                pairs.append((4 * j + 2, 256, 384))
                npair = len(pairs)
                for hl in range(HPC):
                    qT = qk_tiles[hl]
                    kT = qk_tiles[2 + hl]
                    yps = ps_y.tile([128, TQ], F32, tag="y")
                    zacc = z_pool.tile([128, TQ], F32, tag="zacc")
                    e_tiles = [None] * npair

                    def emit_pair(p):
                        i0, c00, c01 = pairs[p]
                        sps = ps_s.tile([128, 2 * TQ], F32, tag="s")
                        for u, c0 in ((0, c00), (1, c01)):
                            i = i0 + u
                            nc.tensor.matmul(
                                sps[:, u * TQ + c0 : (u + 1) * TQ],
                                kT[:, i * TK : (i + 1) * TK],
                                qT[:, j * TQ + c0 : (j + 1) * TQ],
                                start=True,
                                stop=True,
                            )
                        e = e_pool.tile([128, 2 * TQ], BF16, tag="e")
                        # one exp over [c00 : 1024]; the gap columns
                        # [TQ : TQ+c01) hold garbage that is never read.
                        nc.scalar.activation(
                            e[:, c00:], sps[:, c00:], EXP, bias=0.0, scale=SCALE
                        )
                        e_tiles[p] = e

                    def emit_consume(p):
                        i0, c00, c01 = pairs[p]
                        e = e_tiles[p]
                        for u, c0 in ((0, c00), (1, c01)):
                            i = i0 + u
                            eh = e[:, u * TQ + c0 : (u + 1) * TQ]
                            if i >= 4 * j:  # diagonal block: triangle mask
                                nc.vector.tensor_mul(
                                    e[:, u * TQ + c0 : u * TQ + c0 + 128],
                                    e[:, u * TQ + c0 : u * TQ + c0 + 128],
                                    tri_sb[:],
                                )
                            if i == 0:
                                nc.vector.tensor_copy(zacc[:], e[:, 0:TQ])
                            else:
                                nc.vector.tensor_add(
                                    zacc[:, c0:], zacc[:, c0:], eh
                                )
                            nc.tensor.matmul(
                                yps[:, c0:],
                                v_sb[:, i, hl * D : (hl + 1) * D],
                                eh,
                                start=(i == 0),
                                stop=(i == nblk - 1),
                            )

                    for p in range(npair):
                        emit_pair(p)
                        if p >= PAIR_LOOKAHEAD:
                            emit_consume(p - PAIR_LOOKAHEAD)
                    for p in range(max(0, npair - PAIR_LOOKAHEAD), npair):
                        emit_consume(p)

                    fin_backlog.append(emit_finalize(yps, zacc, hl, jsl))
                    # drain the previous (j,hl)'s finalize now: its gpsimd
                    # all-reduce has had a full head-slot to complete, so the
                    # DVE queue won't stall on it.
                    if len(fin_backlog) > 1:
                        drain_finalize(*fin_backlog.pop(0))
            while fin_backlog:
                drain_finalize(*fin_backlog.pop(0))
            return yn_h

        def cproj_phase(b, yn_h):
            oq = [nc.sync, nc.gpsimd]
            for mt in range(NTK):
                osb = ob_pool.tile([128, C], BF16, tag="ob")
                for np_ in range(NTQ // 2):
                    ops = ps_s.tile([128, 2 * TQ], F32, tag="s")
                    for u in range(2):
                        n = 2 * np_ + u
                        nsl_ps = slice(u * TQ, (u + 1) * TQ)
                        for hl in range(HPC):
                            nc.tensor.matmul(
                                ops[:, nsl_ps],
                                yn_h[hl][:, mt * 128 : (mt + 1) * 128],
                                wpr_sb[:, hl, n * TQ : (n + 1) * TQ],
                                start=(hl == 0),
                                stop=(hl == HPC - 1),
                            )
                    osl = slice(2 * np_ * TQ, 2 * (np_ + 1) * TQ)
                    if np_ % 2 == 0:
                        nc.vector.tensor_copy(osb[:, osl], ops[:])
                    else:
                        nc.scalar.copy(osb[:, osl], ops[:])
                oq[mt % 2].dma_start(
                    out[b * T + mt * 128 : b * T + (mt + 1) * 128, :], osb[:]
                )

        xt_sb = xt_b0
        for b in range(B):
            qk_tiles, v_sb = qkv_phase(b, xt_sb)
            if b + 1 < B:
                xt_sb = load_xt(b + 1, [nc.sync, nc.gpsimd])
            yn_h = attention(b, qk_tiles, v_sb)
            cproj_phase(b, yn_h)

    nc.compile()
    return nc


# ---- host-side sharding / unsharding ----

def _rope_cos_sin():
    inv_freq = 1.0 / (ROPE_BASE ** (np.arange(0, D, 2, dtype=np.float32) / D))
    t = np.arange(T, dtype=np.float32)
    freqs = np.outer(t, inv_freq).astype(np.float32)
    emb = np.concatenate([freqs, freqs], axis=-1)
    return np.cos(emb).astype(np.float32), np.sin(emb).astype(np.float32)


def _tri():
    a = np.arange(128)[:, None]
    c = np.arange(128)[None, :]
    return (a <= c).astype(np.float32).astype(ml_dtypes.bfloat16)


_PROGRAM_CACHE = {}


def _get_program(with_bias_qk, with_bias_v):
    key = (with_bias_qk, with_bias_v)
    if key not in _PROGRAM_CACHE:
        _PROGRAM_CACHE[key] = _build_program(with_bias_qk, with_bias_v)
    return _PROGRAM_CACHE[key]


def _make_in_maps(x, W_attn, b_attn, W_proj):
    bf = ml_dtypes.bfloat16
    x = np.asarray(x, dtype=np.float32)
    W_attn = np.asarray(W_attn, dtype=np.float32)
    b_attn = np.asarray(b_attn, dtype=np.float32)
    W_proj = np.asarray(W_proj, dtype=np.float32)

    xT = np.ascontiguousarray(
        x.transpose(2, 0, 1).reshape(C, BT)
    ).astype(bf)
    Wq, Wk, Wv = W_attn[:, :C], W_attn[:, C : 2 * C], W_attn[:, 2 * C :]
    bq, bk, bvv = b_attn[:C], b_attn[C : 2 * C], b_attn[2 * C :]
    cos, sin = _rope_cos_sin()
    cosT = np.ascontiguousarray(cos.T)
    sinNT = np.ascontiguousarray(sin.T).copy()
    sinNT[:HD, :] *= -1.0  # sign-folded for the rotate_half DVE trick
    tri = _tri()

    in_maps = []
    for c in range(N_CORES):
        h0, h1 = HPC * c, HPC * c + 1
        sl0, sl1 = slice(h0 * D, (h0 + 1) * D), slice(h1 * D, (h1 + 1) * D)
        wqk_c = np.concatenate(
            [Wq[:, sl0], Wq[:, sl1], Wk[:, sl0], Wk[:, sl1]], axis=1
        ).astype(bf).reshape(NCT, 128, 4 * D).transpose(1, 0, 2)
        wv_c = (np.concatenate([Wv[:, sl0], Wv[:, sl1]], axis=1)
                .astype(bf).reshape(NCT, 128, HPC * D).transpose(1, 0, 2))
        wpr_c = (np.concatenate([W_proj[sl0, :], W_proj[sl1, :]], axis=0)
                 .astype(bf).reshape(HPC, 128, C).transpose(1, 0, 2))
        bqk_c = np.concatenate([bq[sl0], bq[sl1], bk[sl0], bk[sl1]]).astype(
            np.float32
        ).reshape(4, 128).T
        bv_c = np.concatenate([bvv[sl0], bvv[sl1]]).astype(np.float32)
        in_maps.append(
            {
                "xT": xT,
                "wqk": np.ascontiguousarray(wqk_c),
                "wv": np.ascontiguousarray(wv_c),
                "wpr": np.ascontiguousarray(wpr_c),
                "bqk": np.ascontiguousarray(bqk_c),
                "bqkr": np.ascontiguousarray(
                    np.concatenate([bqk_c[64:], bqk_c[:64]], axis=0)
                ),
                "bv": bv_c,
                "cosT": cosT,
                "sinNT": sinNT,
                "tri": tri,
            }
        )
    return in_maps


def _ensure_ntff_hook():
    """Bridge the missing antenv.axon_hooks module so trace=True can profile.

    The axon boot code registers an NTFF profiling hook via
    antenv.axon_hooks, which this image's antenv package lacks. Install a
    minimal in-memory module and register the ctypes-based hook from
    trn_agent_boot. Only used for profiling runs; best-effort.
    """
    import types

    if "antenv.axon_hooks" in sys.modules:
        return
    try:
        import antenv

        mod = types.ModuleType("antenv.axon_hooks")
        holder = {"hook": None}
        mod.set_axon_ntff_profile_hook = lambda h: holder.__setitem__("hook", h)
        mod.get_axon_ntff_profile_hook = lambda: holder["hook"]
        sys.modules["antenv.axon_hooks"] = mod
        antenv.axon_hooks = mod
        axon_site = "/root/.axon_site"
        if os.path.isdir(axon_site) and axon_site not in sys.path:
            sys.path.insert(0, axon_site)
        from trn_agent_boot.trn_boot import _ntff_profile_via_ctypes

        hook = _ntff_profile_via_ctypes("/opt/axon/libaxon_pjrt.so")
        if hook is not None:
            mod.set_axon_ntff_profile_hook(hook)
    except Exception as e:  # profiling is best-effort
        print(f"[ntff hook unavailable: {type(e).__name__}: {e}]", flush=True)


def run(x, W_attn, b_attn, W_proj, b_proj, trace=False):
    if trace:
        _ensure_ntff_hook()
        import concourse.bass_utils as _bu

        _bu.upload_artifacts = lambda tmpdir: f"local://{tmpdir}"
    b_attn = np.asarray(b_attn, dtype=np.float32)
    b_proj = np.asarray(b_proj, dtype=np.float32)
    with_bias_qk = bool(np.any(b_attn[: 2 * C] != 0.0))
    with_bias_v = bool(np.any(b_attn[2 * C :] != 0.0))
    nc = _get_program(with_bias_qk, with_bias_v)
    in_maps = _make_in_maps(x, W_attn, b_attn, W_proj)
    res = run_bass_kernel_spmd(
        nc, in_maps, list(range(N_CORES)), trace=trace
    )
    acc = np.zeros((BT, C), dtype=np.float32)
    for r in res.results:
        acc += np.asarray(r["out"], dtype=np.float32)
    acc += b_proj[None, :]
    return acc.reshape(B, T, C).astype(np.float32), res


def kernel(x, W_attn, b_attn, W_proj, b_proj):
    out, _ = run(x, W_attn, b_attn, W_proj, b_proj, trace=False)
    return out


# revision 27
# speedup vs baseline: 1.1181x; 1.0270x over previous
"""Trainium2 Bass kernel for causal self-attention with RoPE (tensor-parallel over 8 cores).

Contract: kernel(**inputs) takes full unsharded inputs (x, W_attn, b_attn,
W_proj, b_proj), shards across 8 NeuronCores (2 heads each), runs one SPMD
Bass/Tile kernel, and host-reduces the partial c_proj outputs.

Design notes (HW-measured best of 10 structural variants, ~433us vs 452us
baseline on core 0):
- RoPE entirely on DVE via partition-shifted reads of the chain psum with
  a sign-folded sin table (replaces 64 rotation matmuls + 32 scalar
  copies of the baseline).
- Causal column restriction: diagonal key-blocks only compute score/exp/
  attV/Z columns >= c0; a single shared [128,128] triangle mask handles
  the block-diagonal boundary (~15% less attention work than full-block).
- Softmax denominator Z accumulated on DVE (f32), finalized with a gpsimd
  partition_all_reduce (replaces 160 [1,512] PE matmuls, ~59us of PE);
  1/Z fused into the y-psum evacuation via scalar_tensor_tensor.
- Heads interleaved per q-tile with double-buffered y-PSUM.
- PSUM: qkv/v chains 2 banks, score pairs 2x[128,1024] 4 banks, y 2.
"""

import os
import sys

import numpy as np

for _p in ("/opt/trn_rl_repo",):
    if os.path.isdir(_p) and _p not in sys.path:
        sys.path.insert(0, _p)

import ml_dtypes
from contextlib import ExitStack

import concourse.bass as bass
import concourse.tile as tile
from concourse import bacc, bass_isa, mybir
from concourse.bass_utils import run_bass_kernel_spmd

# ---- problem constants (hardcoded per contract) ----
B, T, C = 2, 2048, 2048
H, D = 16, 128
N_CORES = 8
HPC = H // N_CORES  # heads per core = 2
ROPE_BASE = 10000.0
SCALE = float(1.0 / np.sqrt(D))
TQ = 512            # query tile (free dim of scores matmul)
NTQ = T // TQ       # 4
TK = 128            # key tile (partition dim of scoresT)
NTK = T // TK       # 16
NCT = C // 128      # 16 contraction tiles for projections
BT = B * T
HD = D // 2         # rope half

F32 = mybir.dt.float32
BF16 = mybir.dt.bfloat16

ADD = mybir.AluOpType.add
MULT = mybir.AluOpType.mult
EXP = mybir.ActivationFunctionType.Exp

PAIR_LOOKAHEAD = 2  # score-pairs ahead of attV in the attention pipeline


def _build_program(with_bias_qk: bool, with_bias_v: bool):
    nc = bacc.Bacc(
        "TRN2", target_bir_lowering=False, debug=False, num_devices=N_CORES
    )

    xT = nc.dram_tensor("xT", [C, BT], BF16, kind="ExternalInput").ap()
    wqk = nc.dram_tensor("wqk", [128, NCT, 4 * D], BF16, kind="ExternalInput").ap()
    wv = nc.dram_tensor("wv", [128, NCT, HPC * D], BF16, kind="ExternalInput").ap()
    wpr = nc.dram_tensor("wpr", [128, HPC, C], BF16, kind="ExternalInput").ap()
    bqk = nc.dram_tensor("bqk", [128, 4], F32, kind="ExternalInput").ap()
    bqkr = nc.dram_tensor("bqkr", [128, 4], F32, kind="ExternalInput").ap()
    bv = nc.dram_tensor("bv", [HPC * D], F32, kind="ExternalInput").ap()
    cosT = nc.dram_tensor("cosT", [D, T], F32, kind="ExternalInput").ap()
    sinNT = nc.dram_tensor("sinNT", [D, T], F32, kind="ExternalInput").ap()
    tri = nc.dram_tensor("tri", [128, 128], BF16, kind="ExternalInput").ap()
    out = nc.dram_tensor("out", [BT, C], BF16, kind="ExternalOutput").ap()

    with tile.TileContext(nc) as tc, ExitStack() as ctx:
        consts = ctx.enter_context(tc.tile_pool(name="consts", bufs=1))
        xt_pool = ctx.enter_context(tc.tile_pool(name="xt", bufs=1))
        qk_pool = ctx.enter_context(tc.tile_pool(name="qk", bufs=1))
        v_pool = ctx.enter_context(tc.tile_pool(name="v", bufs=1))
        e_pool = ctx.enter_context(tc.tile_pool(name="e", bufs=6))
        r_pool = ctx.enter_context(tc.tile_pool(name="rp", bufs=2))
        z_pool = ctx.enter_context(tc.tile_pool(name="zs", bufs=3))
        yn_pool = ctx.enter_context(tc.tile_pool(name="yn", bufs=1))
        ob_pool = ctx.enter_context(tc.tile_pool(name="ob", bufs=3))
        ps_mm = ctx.enter_context(tc.tile_pool(name="ps_mm", bufs=2, space="PSUM"))
        ps_s = ctx.enter_context(tc.tile_pool(name="ps_s", bufs=2, space="PSUM"))
        ps_y = ctx.enter_context(tc.tile_pool(name="ps_y", bufs=2, space="PSUM"))

        # ---- initial loads: wqk in 3 chunks across the 3 DMA-capable queues,
        # then x strips round-robin ----
        qs = [nc.sync, nc.gpsimd, nc.scalar]
        wqk_sb = consts.tile([128, NCT, 4 * D], BF16)
        for i, sl in enumerate((slice(0, 5), slice(5, 10), slice(10, 16))):
            qs[i].dma_start(wqk_sb[:, sl, :], wqk[:, sl, :])

        def load_xt(b, queues):
            xt_sb = xt_pool.tile([128, NCT, T], BF16, tag="xt")
            for ct in range(NCT):
                queues[ct % len(queues)].dma_start(
                    xt_sb[:, ct, :],
                    xT[ct * 128 : (ct + 1) * 128, b * T : (b + 1) * T],
                )
            return xt_sb

        xt_b0 = load_xt(0, qs)

        cos_sb = consts.tile([128, T], F32)
        nc.sync.dma_start(cos_sb[:], cosT[:])
        sin_sb = consts.tile([128, T], F32)
        nc.gpsimd.dma_start(sin_sb[:], sinNT[:])
        tri_sb = consts.tile([128, 128], BF16)
        nc.scalar.dma_start(tri_sb[:], tri[:])
        wv_sb = consts.tile([128, NCT, HPC * D], BF16)
        nc.scalar.dma_start(wv_sb[:], wv[:])
        wpr_sb = consts.tile([128, HPC, C], BF16)
        nc.sync.dma_start(wpr_sb[:], wpr[:])
        if with_bias_qk:
            bqk_sb = consts.tile([128, 4], F32)
            nc.gpsimd.dma_start(bqk_sb[:], bqk[:])
        if with_bias_v:
            bv_sb = consts.tile([128, HPC * D], F32)
            nc.gpsimd.dma_start(bv_sb[:], bv.to_broadcast((128, HPC * D)))

        def emit_rope(f, t, w, ps, qk_tiles):
            """Matmul-free rope over w cols starting at q-tile t:
            qk[f][:, tsl] = (q+b)*cos + rot_half(q+b)*sinN.
            All four passes run on DVE; the shifted-base reads are legal
            because in0 is PSUM."""
            tsl = slice(t * TQ, t * TQ + w)
            b_all = bqk_sb[:, f : f + 1] if with_bias_qk else 0.0
            b_lo = bqk_sb[0:HD, f : f + 1] if with_bias_qk else 0.0
            b_hi = bqk_sb[HD:D, f : f + 1] if with_bias_qk else 0.0
            t1 = r_pool.tile([128, 2 * TQ], F32, tag="r1")
            nc.vector.scalar_tensor_tensor(
                t1[:, 0:w], ps[:, 0:w], b_all, cos_sb[:, tsl], op0=ADD, op1=MULT
            )
            t2 = r_pool.tile([128, 2 * TQ], F32, tag="r2")
            nc.vector.scalar_tensor_tensor(
                t2[0:HD, 0:w], ps[HD:D, 0:w], b_hi, sin_sb[0:HD, tsl],
                op0=ADD, op1=MULT,
            )
            nc.vector.scalar_tensor_tensor(
                t2[HD:D, 0:w], ps[0:HD, 0:w], b_lo, sin_sb[HD:D, tsl],
                op0=ADD, op1=MULT,
            )
            nc.vector.tensor_add(qk_tiles[f][:, tsl], t1[:, 0:w], t2[:, 0:w])

        def qkv_phase(b, xt_sb):
            """QKV projections + RoPE for batch b. Returns (qk_tiles, v_sb)."""
            # q/k feature tiles: 0=q_h0, 1=q_h1, 2=k_h0, 3=k_h1
            qk_tiles = [
                qk_pool.tile([128, T], BF16, tag=f"qk{f}", name=f"qkt{f}")
                for f in range(4)
            ]
            if b == 0:
                # cold start: t=0 for all four f-tiles ct-major so the PE
                # consumes xT strips as the initial DMAs land.
                cold_a = ps_s.tile([128, 2 * TQ], F32, tag="s", name="cold_a")
                cold_b = ps_s.tile([128, 2 * TQ], F32, tag="s", name="cold_b")
                t0_ps = [
                    cold_a[:, 0:TQ], cold_a[:, TQ : 2 * TQ],
                    cold_b[:, 0:TQ], cold_b[:, TQ : 2 * TQ],
                ]
                for ct in range(NCT):
                    for f in range(4):
                        nc.tensor.matmul(
                            t0_ps[f],
                            wqk_sb[:, ct, f * 128 : (f + 1) * 128],
                            xt_sb[:, ct, 0:TQ],
                            start=(ct == 0),
                            stop=(ct == NCT - 1),
                        )
                for f in range(4):
                    emit_rope(f, 0, TQ, t0_ps[f], qk_tiles)
            for f in range(4):
                for t in range(NTQ):
                    if b == 0 and t == 0:
                        continue
                    ps = ps_mm.tile([128, TQ], F32, tag="mm")
                    for ct in range(NCT):
                        nc.tensor.matmul(
                            ps[:],
                            wqk_sb[:, ct, f * 128 : (f + 1) * 128],
                            xt_sb[:, ct, t * TQ : (t + 1) * TQ],
                            start=(ct == 0),
                            stop=(ct == NCT - 1),
                        )
                    emit_rope(f, t, TQ, ps, qk_tiles)

            # V in [t, d] layout: lhsT = xT tile (c, t), rhs = Wv (c, d)
            v_sb = v_pool.tile([128, NTK, HPC * D], BF16, tag="v")
            for mt in range(NTK):
                ps = ps_mm.tile([128, HPC * D], F32, tag="mm")
                for ct in range(NCT):
                    nc.tensor.matmul(
                        ps[:],
                        xt_sb[:, ct, mt * 128 : (mt + 1) * 128],
                        wv_sb[:, ct, :],
                        start=(ct == 0),
                        stop=(ct == NCT - 1),
                    )
                if with_bias_v:
                    nc.vector.tensor_add(v_sb[:, mt, :], ps[:], bv_sb[:])
                else:
                    nc.scalar.copy(v_sb[:, mt, :], ps[:])
            return qk_tiles, v_sb

        def attention(b, qk_tiles, v_sb):
            """Flash-style causal attention, heads interleaved per q-tile.

            Returns yn tiles ([d, T] bf16, one per head)."""
            yn_h = [
                yn_pool.tile([128, T], BF16, tag=f"yn{hl}", name=f"yn{hl}")
                for hl in range(HPC)
            ]
            fin_backlog = []

            def emit_finalize(yps, zacc, hl, jsl):
                zsum = z_pool.tile([128, TQ], F32, tag="zsum", bufs=2)
                nc.gpsimd.partition_all_reduce(
                    zsum[:], zacc[:], channels=128, reduce_op=bass_isa.ReduceOp.add
                )
                return (yps, zsum, hl, jsl)

            def drain_finalize(yps, zsum, hl, jsl):
                zrec = z_pool.tile([128, TQ], F32, tag="zrec", bufs=2)
                nc.vector.reciprocal_approx_fast(zrec[:], zsum[:])
                nc.vector.scalar_tensor_tensor(
                    yn_h[hl][:, jsl], yps[:], 0.0, zrec[:], op0=ADD, op1=MULT
                )

            for j in range(NTQ):
                jsl = slice(j * TQ, (j + 1) * TQ)
                nblk = 4 * j + 4
                # pairs of key-blocks: (i0, c0_of_i0, c0_of_i1); c0 = first
                # valid scores column (block-local) for causality.
                pairs = [(2 * p, 0, 0) for p in range(2 * j)]
                pairs.append((4 * j, 0, 128))
                pairs.append((4 * j + 2, 256, 384))
                npair = len(pairs)
                for hl in range(HPC):
                    qT = qk_tiles[hl]
                    kT = qk_tiles[2 + hl]
                    yps = ps_y.tile([128, TQ], F32, tag="y")
                    zacc = z_pool.tile([128, TQ], F32, tag="zacc")
                    e_tiles = [None] * npair

                    def emit_pair(p):
                        i0, c00, c01 = pairs[p]
                        sps = ps_s.tile([128, 2 * TQ], F32, tag="s")
                        for u, c0 in ((0, c00), (1, c01)):
                            i = i0 + u
                            nc.tensor.matmul(
                                sps[:, u * TQ + c0 : (u + 1) * TQ],
                                kT[:, i * TK : (i + 1) * TK],
                                qT[:, j * TQ + c0 : (j + 1) * TQ],
                                start=True,
                                stop=True,
                            )
                        e = e_pool.tile([128, 2 * TQ], BF16, tag="e")
                        # one exp over [c00 : 1024]; the gap columns
                        # [TQ : TQ+c01) hold garbage that is never read.
                        nc.scalar.activation(
                            e[:, c00:], sps[:, c00:], EXP, bias=0.0, scale=SCALE
                        )
                        e_tiles[p] = e

                    def emit_consume(p):
                        i0, c00, c01 = pairs[p]
                        e = e_tiles[p]
                        for u, c0 in ((0, c00), (1, c01)):
                            i = i0 + u
                            eh = e[:, u * TQ + c0 : (u + 1) * TQ]
                            if i >= 4 * j:  # diagonal block: triangle mask
                                nc.vector.tensor_mul(
                                    e[:, u * TQ + c0 : u * TQ + c0 + 128],
                                    e[:, u * TQ + c0 : u * TQ + c0 + 128],
                                    tri_sb[:],
                                )
                            if i == 0:
                                nc.vector.tensor_copy(zacc[:], e[:, 0:TQ])
                            else:
                                nc.vector.tensor_add(
                                    zacc[:, c0:], zacc[:, c0:], eh
                                )
                            nc.tensor.matmul(
                                yps[:, c0:],
                                v_sb[:, i, hl * D : (hl + 1) * D],
                                eh,
                                start=(i == 0),
                                stop=(i == nblk - 1),
                            )

                    for p in range(npair):
                        emit_pair(p)
                        if p >= PAIR_LOOKAHEAD:
                            emit_consume(p - PAIR_LOOKAHEAD)
                    for p in range(max(0, npair - PAIR_LOOKAHEAD), npair):
                        emit_consume(p)

                    fin_backlog.append(emit_finalize(yps, zacc, hl, jsl))
                    # drain the previous (j,hl)'s finalize now: its gpsimd
                    # all-reduce has had a full head-slot to complete, so the
                    # DVE queue won't stall on it.
                    if len(fin_backlog) > 1:
                        drain_finalize(*fin_backlog.pop(0))
            while fin_backlog:
                drain_finalize(*fin_backlog.pop(0))
            return yn_h

        def cproj_phase(b, yn_h):
            oq = [nc.sync, nc.gpsimd]
            for mt in range(NTK):
                osb = ob_pool.tile([128, C], BF16, tag="ob")
                for np_ in range(NTQ // 2):
                    ops = ps_s.tile([128, 2 * TQ], F32, tag="s")
                    for u in range(2):
                        n = 2 * np_ + u
                        nsl_ps = slice(u * TQ, (u + 1) * TQ)
                        for hl in range(HPC):
                            nc.tensor.matmul(
                                ops[:, nsl_ps],
                                yn_h[hl][:, mt * 128 : (mt + 1) * 128],
                                wpr_sb[:, hl, n * TQ : (n + 1) * TQ],
                                start=(hl == 0),
                                stop=(hl == HPC - 1),
                            )
                    osl = slice(2 * np_ * TQ, 2 * (np_ + 1) * TQ)
                    if np_ % 2 == 0:
                        nc.vector.tensor_copy(osb[:, osl], ops[:])
                    else:
                        nc.scalar.copy(osb[:, osl], ops[:])
                oq[mt % 2].dma_start(
                    out[b * T + mt * 128 : b * T + (mt + 1) * 128, :], osb[:]
                )

        xt_sb = xt_b0
        for b in range(B):
            qk_tiles, v_sb = qkv_phase(b, xt_sb)
            if b + 1 < B:
                xt_sb = load_xt(b + 1, [nc.sync, nc.gpsimd])
            yn_h = attention(b, qk_tiles, v_sb)
            cproj_phase(b, yn_h)

    nc.compile()
    return nc


# ---- host-side sharding / unsharding ----

def _rope_cos_sin():
    inv_freq = 1.0 / (ROPE_BASE ** (np.arange(0, D, 2, dtype=np.float32) / D))
    t = np.arange(T, dtype=np.float32)
    freqs = np.outer(t, inv_freq).astype(np.float32)
    emb = np.concatenate([freqs, freqs], axis=-1)
    return np.cos(emb).astype(np.float32), np.sin(emb).astype(np.float32)


def _tri():
    a = np.arange(128)[:, None]
    c = np.arange(128)[None, :]
    return (a <= c).astype(np.float32).astype(ml_dtypes.bfloat16)


_PROGRAM_CACHE = {}


def _get_program(with_bias_qk, with_bias_v):
    key = (with_bias_qk, with_bias_v)
    if key not in _PROGRAM_CACHE:
        _PROGRAM_CACHE[key] = _build_program(with_bias_qk, with_bias_v)
    return _PROGRAM_CACHE[key]


def _make_in_maps(x, W_attn, b_attn, W_proj):
    bf = ml_dtypes.bfloat16
    x = np.asarray(x, dtype=np.float32)
    W_attn = np.asarray(W_attn, dtype=np.float32)
    b_attn = np.asarray(b_attn, dtype=np.float32)
    W_proj = np.asarray(W_proj, dtype=np.float32)

    xT = np.ascontiguousarray(
        x.transpose(2, 0, 1).reshape(C, BT)
    ).astype(bf)
    Wq, Wk, Wv = W_attn[:, :C], W_attn[:, C : 2 * C], W_attn[:, 2 * C :]
    bq, bk, bvv = b_attn[:C], b_attn[C : 2 * C], b_attn[2 * C :]
    cos, sin = _rope_cos_sin()
    cosT = np.ascontiguousarray(cos.T)
    sinNT = np.ascontiguousarray(sin.T).copy()
    sinNT[:HD, :] *= -1.0  # sign-folded for the rotate_half DVE trick
    tri = _tri()

    in_maps = []
    for c in range(N_CORES):
        h0, h1 = HPC * c, HPC * c + 1
        sl0, sl1 = slice(h0 * D, (h0 + 1) * D), slice(h1 * D, (h1 + 1) * D)
        wqk_c = np.concatenate(
            [Wq[:, sl0], Wq[:, sl1], Wk[:, sl0], Wk[:, sl1]], axis=1
        ).astype(bf).reshape(NCT, 128, 4 * D).transpose(1, 0, 2)
        wv_c = (np.concatenate([Wv[:, sl0], Wv[:, sl1]], axis=1)
                .astype(bf).reshape(NCT, 128, HPC * D).transpose(1, 0, 2))
        wpr_c = (np.concatenate([W_proj[sl0, :], W_proj[sl1, :]], axis=0)
                 .astype(bf).reshape(HPC, 128, C).transpose(1, 0, 2))
        bqk_c = np.concatenate([bq[sl0], bq[sl1], bk[sl0], bk[sl1]]).astype(
            np.float32
        ).reshape(4, 128).T
        bv_c = np.concatenate([bvv[sl0], bvv[sl1]]).astype(np.float32)
        in_maps.append(
            {
                "xT": xT,
                "wqk": np.ascontiguousarray(wqk_c),
                "wv": np.ascontiguousarray(wv_c),
                "wpr": np.ascontiguousarray(wpr_c),
                "bqk": np.ascontiguousarray(bqk_c),
                "bqkr": np.ascontiguousarray(
                    np.concatenate([bqk_c[64:], bqk_c[:64]], axis=0)
                ),
                "bv": bv_c,
                "cosT": cosT,
                "sinNT": sinNT,
                "tri": tri,
            }
        )
    return in_maps


def _ensure_ntff_hook():
    """Bridge the missing antenv.axon_hooks module so trace=True can profile.

    The axon boot code registers an NTFF profiling hook via
    antenv.axon_hooks, which this image's antenv package lacks. Install a
    minimal in-memory module and register the ctypes-based hook from
    trn_agent_boot. Only used for profiling runs; best-effort.
    """
    import types

    if "antenv.axon_hooks" in sys.modules:
        return
    try:
        import antenv

        mod = types.ModuleType("antenv.axon_hooks")
        holder = {"hook": None}
        mod.set_axon_ntff_profile_hook = lambda h: holder.__setitem__("hook", h)
        mod.get_axon_ntff_profile_hook = lambda: holder["hook"]
        sys.modules["antenv.axon_hooks"] = mod
        antenv.axon_hooks = mod
        axon_site = "/root/.axon_site"
        if os.path.isdir(axon_site) and axon_site not in sys.path:
            sys.path.insert(0, axon_site)
        from trn_agent_boot.trn_boot import _ntff_profile_via_ctypes

        hook = _ntff_profile_via_ctypes("/opt/axon/libaxon_pjrt.so")
        if hook is not None:
            mod.set_axon_ntff_profile_hook(hook)
    except Exception as e:  # profiling is best-effort
        print(f"[ntff hook unavailable: {type(e).__name__}: {e}]", flush=True)


def run(x, W_attn, b_attn, W_proj, b_proj, trace=False):
    if trace:
        _ensure_ntff_hook()
        import concourse.bass_utils as _bu

        _bu.upload_artifacts = lambda tmpdir: f"local://{tmpdir}"
    b_attn = np.asarray(b_attn, dtype=np.float32)
    b_proj = np.asarray(b_proj, dtype=np.float32)
    with_bias_qk = bool(np.any(b_attn[: 2 * C] != 0.0))
    with_bias_v = bool(np.any(b_attn[2 * C :] != 0.0))
    nc = _get_program(with_bias_qk, with_bias_v)
    in_maps = _make_in_maps(x, W_attn, b_attn, W_proj)
    res = run_bass_kernel_spmd(
        nc, in_maps, list(range(N_CORES)), trace=trace
    )
    acc = np.zeros((BT, C), dtype=np.float32)
    for r in res.results:
        acc += np.asarray(r["out"], dtype=np.float32)
    acc += b_proj[None, :]
    return acc.reshape(B, T, C).astype(np.float32), res


def kernel(x, W_attn, b_attn, W_proj, b_proj):
    out, _ = run(x, W_attn, b_attn, W_proj, b_proj, trace=False)
    return out
